# revision 18
# baseline (speedup 1.0000x reference)
"""Trainium2 Bass kernel: 16-head RoPE attention block (B=4, T=2048, D=2048).

Sharding: tensor-parallel over heads. Each of the 8 cores owns 2 heads
(a 256-wide slice of the q/k/v projection output features). Per core:

  stage 1: q/k/v projections in feature-major layout, all-bf16 matmuls
           (x and W arrive bf16: halves DMA + SBUF traffic at the same
           1-col/cycle PE rate), RoPE on the vector engine (bf16 out),
           v transposed to token-major via the PE; results staged in
           DRAM per (head, batch). Chunk 0 is quarter-reordered (q/k
           groups track the x quarters as they land; v groups follow)
           with the loads spread over the three DMA rings.
  stage 2: per (batch, head): scores computed TRANSPOSED (S^T[k,q] =
           kTile^T @ qT, bf16) in PAIRS of k-tiles sharing one two-bank
           PSUM tile so each scalar-engine EXP covers 1024 columns (the
           352-cycle ACT overhead amortizes and the scalar engine stays
           below the PE; only the exp table set is ever loaded). The
           softmax denominator accumulates as bf16 pair-sums + a running
           bf16 acc on the vector engine, is collapsed to a [1,qch] row
           by a ones-column matmul, bounced through DRAM into a [128,4]
           layout for a full-lane DVE reciprocal, and broadcast back
           with a 0-stride DMA read. The ~10us of bounce latency is
           hidden: out-projection work referencing a chunk is deferred
           by one query chunk.
  stage 3: out-projection (bf16 x bf16) matmuls INTERLEAVED into the
           attention loop one query-chunk later, filling the PE's
           exp-wait gaps; evacuated to bf16 (3:1 vector/scalar split)
           and DMAed out on the sync ring.

Host sums the 8 bf16 partial outputs (the "all-reduce") in f32 and
un-transposes. PSUM budget: score pairs 2x2 banks + pv 2 + s3/dnrow 2
= 8 banks. Stage-2 input q/k/v tiles live in a right-side SBUF pool
that outlives stage 1, so batch 0 prefetches with no WAR stall at the
stage boundary. Measured: 902.7us, rel err 5.9e-3 (vs 1067.1us
baseline).
"""

import math

import numpy as np

import concourse.bacc as bacc
import concourse.bass as bass
import concourse.mybir as mybir
import concourse.tile as tile
from concourse.bass_utils import run_bass_kernel_spmd

F32 = mybir.dt.float32
F32R = mybir.dt.float32r
BF16 = mybir.dt.bfloat16
EXP = mybir.ActivationFunctionType.Exp
LN = mybir.ActivationFunctionType.Ln

# Problem shape (hardcoded; the harness calls kernel() with exactly these).
B = 4
T = 2048
D_MODEL = 2048
HEAD_DIM = 128
N_CORES = 8
ROPE_BASE = 10000.0

HPC = 2                      # heads per core
F_LOC = HPC * HEAD_DIM       # 256 local projection features per core
BT = B * T
TCH = 512                    # token chunk width (stages 1/3)
QCH = 512                    # query chunk width (stage 2)
SCALE = 1.0 / math.sqrt(HEAD_DIM)


def build_module(b=B, t=T, d_model=D_MODEL, n_cores=N_CORES):
    """Build the per-core Bass module. All cores run the same program on
    different data (pure SPMD, no collectives)."""
    bt = b * t
    dt_ = d_model // 128
    kt = t // 128
    npr = kt // 2            # k-tile pairs per query chunk
    tch = min(TCH, bt)
    qch = min(QCH, t)
    ntch = bt // tch
    nqc = t // qch
    cpb = t // tch           # stage-1/3 token chunks per batch
    qd = dt_ // 4            # d-tile quarter for the startup loads

    nc = bacc.Bacc(None, target_bir_lowering=False)

    xT = nc.dram_tensor("xT", [d_model, bt], BF16, kind="ExternalInput")
    wqT = nc.dram_tensor("wqT", [d_model, F_LOC], BF16, kind="ExternalInput")
    wkT = nc.dram_tensor("wkT", [d_model, F_LOC], BF16, kind="ExternalInput")
    wvT = nc.dram_tensor("wvT", [d_model, F_LOC], BF16, kind="ExternalInput")
    woT = nc.dram_tensor("woT", [F_LOC, d_model], BF16, kind="ExternalInput")
    cosT = nc.dram_tensor("cosT", [HEAD_DIM, t], F32, kind="ExternalInput")
    rsinT = nc.dram_tensor("rsinT", [HEAD_DIM, t], F32, kind="ExternalInput")
    ident = nc.dram_tensor("ident", [128, 128], BF16, kind="ExternalInput")
    onesq = nc.dram_tensor("onesq", [128, 128], BF16, kind="ExternalInput")
    outP = nc.dram_tensor("outP", [d_model, bt], BF16, kind="ExternalOutput")

    with tile.TileContext(nc) as tc:
        with (
            tc.tile_pool(name="const", bufs=1) as constp,
            tc.tile_pool(name="dram", bufs=1, space="DRAM") as dram,
            tc.tile_pool(name="psp", bufs=2, space="PSUM") as psp,
        ):
            # constants: tiles here, DMAs emitted inside stage 1 so the
            # per-ring issue order puts the matmul-gating loads first
            cos_sb = constp.tile([128, t], F32)
            rsin_sb = constp.tile([128, t], F32)
            id_sb = constp.tile([128, 128], BF16)
            ones_sb = constp.tile([128, 128], BF16)

            # DRAM scratch, per (head, batch): cross-stage deps stay
            # batch-granular so the stages pipeline
            q_scr = [
                [dram.tile([128, t], BF16, name=f"qs{h}_{bi}", tag=f"qs{h}_{bi}") for bi in range(b)]
                for h in range(HPC)
            ]
            k_scr = [
                [dram.tile([128, t], BF16, name=f"ks{h}_{bi}", tag=f"ks{h}_{bi}") for bi in range(b)]
                for h in range(HPC)
            ]
            v_scr = [
                [dram.tile([kt, 128, 128], BF16, name=f"vs{h}_{bi}", tag=f"vs{h}_{bi}") for bi in range(b)]
                for h in range(HPC)
            ]

            prefetched = {}
            # per-qc scratch rows for the DMA-reshaped reciprocal
            dnrow_d = dram.tile([2, t // 128 // 4, 512], F32, name="dnrow_d")
            rcp_d = dram.tile([2, t // 128 // 4, 512], F32, name="rcp_d")

            # ================= stage 1: projections + rope + v^T =========
            # manual pool lifetimes: the stage-1 pools take the low SBUF
            # addresses; s2in is allocated above them and outlives stage 1
            # (the batch-0 q/k/v prefetch needs no WAR wait on stage-1
            # buffers at the stage boundary)
            wpool = tc.alloc_tile_pool(name="s1w", bufs=1)
            xpool = tc.alloc_tile_pool(name="s1x", bufs=3)
            tpool = tc.alloc_tile_pool(name="s1t", bufs=4)
            s2in = tc.alloc_tile_pool(name="s2in", bufs=2, side="right")
            if True:
                w_sbs = []
                wsrcs = []
                for wten, wname in ((wqT, "wq"), (wkT, "wk"), (wvT, "wv")):
                    wsb = wpool.tile([128, dt_, F_LOC], BF16, tag=wname)
                    w_sbs.append(wsb)
                    wsrcs.append(
                        wten[:, :].rearrange("(dt p) f -> p dt f", p=128)
                    )

                # ---- startup loads over the three DMA rings ----
                x0 = xpool.tile([128, dt_, tch], BF16, tag="x")
                x0src = xT[:, 0:tch].rearrange("(dt p) tt -> p dt tt", p=128)
                # sync ring: the four x chunk-0 quarters (chunks 2+ follow)
                for qtr in range(4):
                    dsl = slice(qtr * qd, (qtr + 1) * qd)
                    nc.sync.dma_start(out=x0[:, dsl, :], in_=x0src[:, dsl, :])
                # scalar ring: wq quarters (x chunk 1 + wo follow)
                for qtr in range(4):
                    dsl = slice(qtr * qd, (qtr + 1) * qd)
                    nc.scalar.dma_start(
                        out=w_sbs[0][:, dsl, :], in_=wsrcs[0][:, dsl, :]
                    )
                # gpsimd ring: wk quarters, wv quarters, then constants
                for wi in (1, 2):
                    for qtr in range(4):
                        dsl = slice(qtr * qd, (qtr + 1) * qd)
                        nc.gpsimd.dma_start(
                            out=w_sbs[wi][:, dsl, :], in_=wsrcs[wi][:, dsl, :]
                        )
                nc.gpsimd.dma_start(out=cos_sb, in_=cosT[:, :])
                nc.gpsimd.dma_start(out=rsin_sb, in_=rsinT[:, :])
                nc.gpsimd.dma_start(out=id_sb, in_=ident[:, :])
                nc.gpsimd.dma_start(out=ones_sb, in_=onesq[:, :])

                def evac_qk(pi, ft, ps, bi, off):
                    """RoPE + bf16 store for one q/k psum group."""
                    lsl = slice(off, off + tch)
                    ro = tpool.tile([128, tch], F32, tag="ro")
                    nc.vector.tensor_mul(ro, ps, cos_sb[:, lsl])
                    rt = tpool.tile([128, tch], F32, tag="rt")
                    nc.vector.tensor_mul(rt[0:64], ps[64:128], rsin_sb[0:64, lsl])
                    nc.vector.tensor_mul(rt[64:128], ps[0:64], rsin_sb[64:128, lsl])
                    rs = tpool.tile([128, tch], BF16, tag="rs")
                    nc.vector.tensor_add(rs, ro, rt)
                    scr = q_scr if pi == 0 else k_scr
                    nc.gpsimd.dma_start(out=scr[ft][bi][:, lsl], in_=rs)

                def evac_v(ft, ps, bi, tch_i):
                    """bf16 copy + PE transpose + store for one v group."""
                    vsb = tpool.tile([128, tch], BF16, tag="vs")
                    nc.scalar.copy(vsb, ps)
                    for j in range(tch // 128):
                        pst = psp.tile([128, 128], BF16, tag="s3", name="pst")
                        nc.tensor.transpose(
                            pst, vsb[:, j * 128 : (j + 1) * 128], id_sb
                        )
                        vt = tpool.tile([128, 128], BF16, tag="vt")
                        nc.vector.tensor_copy(vt, pst)
                        nc.gpsimd.dma_start(
                            out=v_scr[ft][bi][
                                (tch_i % cpb) * (tch // 128) + j, :, :
                            ],
                            in_=vt,
                        )

                # ---- chunk 0: q/k groups follow the x quarters; v after ----
                qk_pairs = []
                for pi in range(2):
                    scp = psp.tile([128, 2, tch], F32, tag="sc", name=f"c0qk{pi}")
                    qk_pairs.append(scp)
                v_ps = []
                for ft in range(HPC):
                    vps = psp.tile([128, tch], F32, tag="pv", name=f"c0v{ft}")
                    v_ps.append(vps)
                for dq in range(4):
                    for pi in range(2):
                        for ft in range(HPC):
                            fsl = slice(ft * 128, (ft + 1) * 128)
                            for di in range(dq * qd, (dq + 1) * qd):
                                nc.tensor.matmul(
                                    qk_pairs[pi][:, ft, :],
                                    w_sbs[pi][:, di, fsl],
                                    x0[:, di, :],
                                    start=(di == 0),
                                    stop=(di == dt_ - 1),
                                )
                for ft in range(HPC):
                    fsl = slice(ft * 128, (ft + 1) * 128)
                    for di in range(dt_):
                        nc.tensor.matmul(
                            v_ps[ft],
                            w_sbs[2][:, di, fsl],
                            x0[:, di, :],
                            start=(di == 0),
                            stop=(di == dt_ - 1),
                        )
                for pi in range(2):
                    for ft in range(HPC):
                        evac_qk(pi, ft, qk_pairs[pi][:, ft, :], 0, 0)
                for ft in range(HPC):
                    evac_v(ft, v_ps[ft], 0, 0)

                # ---- chunks 1..ntch-1 ----
                for tch_i in range(1, ntch):
                    bi = tch_i // cpb
                    off = (tch_i % cpb) * tch
                    tsl = slice(tch_i * tch, (tch_i + 1) * tch)
                    x_sb = xpool.tile([128, dt_, tch], BF16, tag="x")
                    xsrc = xT[:, tsl].rearrange("(dt p) tt -> p dt tt", p=128)
                    # chunk 1 rides the scalar ring (free after wq) so it
                    # lands before chunk 0's compute finishes
                    ring = nc.scalar if tch_i == 1 else nc.sync
                    ring.dma_start(out=x_sb, in_=xsrc)
                    for pi in range(2):
                        scp = psp.tile(
                            [128, 2, tch], F32, tag="sc", name=f"qk{tch_i}_{pi}"
                        )
                        for ft in range(HPC):
                            fsl = slice(ft * 128, (ft + 1) * 128)
                            for di in range(dt_):
                                nc.tensor.matmul(
                                    scp[:, ft, :],
                                    w_sbs[pi][:, di, fsl],
                                    x_sb[:, di, :],
                                    start=(di == 0),
                                    stop=(di == dt_ - 1),
                                )
                        for ft in range(HPC):
                            evac_qk(pi, ft, scp[:, ft, :], bi, off)
                    for ft in range(HPC):
                        fsl = slice(ft * 128, (ft + 1) * 128)
                        vps = psp.tile(
                            [128, tch], F32, tag="pv", name=f"v{tch_i}_{ft}"
                        )
                        for di in range(dt_):
                            nc.tensor.matmul(
                                vps,
                                w_sbs[2][:, di, fsl],
                                x_sb[:, di, :],
                                start=(di == 0),
                                stop=(di == dt_ - 1),
                            )
                        evac_v(ft, vps, bi, tch_i)

                    # prefetch batch-0 q/k/v into the pre-opened s2in pool
                    # (disjoint SBUF: no WAR wait on stage-1 buffers at the
                    # stage boundary)
                    if tch_i == cpb - 1:
                        for h in range(HPC):
                            pq = s2in.tile([128, t], BF16, tag="q", name=f"pq{h}")
                            nc.sync.dma_start(out=pq, in_=q_scr[h][0][:, :])
                            pk = s2in.tile([128, t], BF16, tag="k", name=f"pk{h}")
                            nc.sync.dma_start(out=pk, in_=k_scr[h][0][:, :])
                            pv_ = s2in.tile(
                                [128, kt, 128], BF16, tag="v", name=f"pv{h}"
                            )
                            nc.sync.dma_start(
                                out=pv_,
                                in_=v_scr[h][0][:, :, :].rearrange(
                                    "tt p dh -> p tt dh"
                                ),
                            )
                            prefetched[h] = (pq, pk, pv_)

            tpool.release()
            xpool.release()
            wpool.release()

            # ======== stage 2+3: attention + interleaved out-projection ====
            with (
                tc.tile_pool(name="s2", bufs=2) as s2pool,
                tc.tile_pool(name="s2e", bufs=5) as epool,
                tc.tile_pool(name="s3w", bufs=1) as wopool,
                tc.tile_pool(name="s3o", bufs=6) as s3pool,
            ):
                wo_sb = wopool.tile([128, HPC, d_model], BF16, tag="wo")
                nc.scalar.dma_start(
                    out=wo_sb,
                    in_=woT[:, :].rearrange("(ft p) d -> p ft d", p=128),
                )

                # pending out-projection groups: (attn_tile, bi, c4, do)
                s3_pending = []
                s3_count = [0]

                def emit_s3_group():
                    if not s3_pending:
                        return
                    attn_src, bi_src, c4, do = s3_pending.pop(0)
                    off = c4 * tch
                    gsl = slice(bi_src * t + off, bi_src * t + off + tch)
                    ps = psp.tile([128, tch], F32, tag="s3", name="s3ps")
                    for ft in range(HPC):
                        nc.tensor.matmul(
                            ps,
                            wo_sb[:, ft, do * 128 : (do + 1) * 128],
                            attn_src[:, ft, off : off + tch],
                            start=(ft == 0),
                            stop=(ft == HPC - 1),
                        )
                    osb = s3pool.tile([128, tch], BF16, tag="o")
                    if s3_count[0] % 4 == 3:
                        nc.scalar.copy(osb, ps)
                    else:
                        nc.vector.tensor_copy(osb, ps)
                    ring = nc.sync if s3_count[0] % 2 == 0 else nc.gpsimd
                    s3_count[0] += 1
                    ring.dma_start(
                        out=outP[do * 128 : (do + 1) * 128, gsl], in_=osb
                    )

                for bi in range(b):
                    attn_n = s2pool.tile([128, HPC, t], BF16, tag="an")
                    for h in range(HPC):
                        if bi == 0:
                            q_sb, k_sb, v_sb = prefetched[h]
                        else:
                            q_sb = s2in.tile([128, t], BF16, tag="q")
                            nc.sync.dma_start(out=q_sb, in_=q_scr[h][bi][:, :])
                            k_sb = s2in.tile([128, t], BF16, tag="k")
                            nc.sync.dma_start(out=k_sb, in_=k_scr[h][bi][:, :])
                            v_sb = s2in.tile([128, kt, 128], BF16, tag="v")
                            nc.sync.dma_start(
                                out=v_sb,
                                in_=v_scr[h][bi][:, :, :].rearrange(
                                    "tt p dh -> p tt dh"
                                ),
                            )
                        pending_mul = []
                        for qc in range(nqc):
                            qsl = slice(qc * qch, (qc + 1) * qch)
                            e_pairs = [None] * npr

                            def emit_pair(p):
                                sps = psp.tile(
                                    [128, 2, qch], F32, tag="sc", name="sps"
                                )
                                for j in range(2):
                                    kti = 2 * p + j
                                    nc.tensor.matmul(
                                        sps[:, j, :],
                                        k_sb[:, kti * 128 : (kti + 1) * 128],
                                        q_sb[:, qsl],
                                        start=True,
                                        stop=True,
                                    )
                                e_p = epool.tile(
                                    [128, 2, qch], BF16, tag="E", name="e_p"
                                )
                                # one EXP over both k-tiles (1024 cols):
                                # amortizes the 352-cycle ACT overhead
                                nc.scalar.activation(e_p, sps, EXP, scale=SCALE)
                                e_pairs[p] = e_p

                            emit_pair(0)
                            emit_pair(1)
                            while pending_mul:
                                m_pv, m_rcp, m_sl = pending_mul.pop(0)
                                nc.vector.tensor_mul(
                                    attn_n[:, h, m_sl], m_pv, m_rcp
                                )
                            pv = psp.tile([128, qch], F32, tag="pv", name="pv")
                            acc = s2pool.tile(
                                [128, qch], BF16, tag="acc", name="acc"
                            )
                            for p in range(npr):
                                for j in range(2):
                                    nc.tensor.matmul(
                                        pv,
                                        v_sb[:, 2 * p + j, :],
                                        e_pairs[p][:, j, :],
                                        start=(p == 0 and j == 0),
                                        stop=(p == npr - 1 and j == 1),
                                    )
                                # denominator: bf16 pair-collapse (gpsimd)
                                # + running bf16 acc (vector)
                                if p == 0:
                                    nc.vector.tensor_add(
                                        acc,
                                        e_pairs[0][:, 0, :],
                                        e_pairs[0][:, 1, :],
                                    )
                                else:
                                    sp = s2pool.tile(
                                        [128, qch], BF16, tag="sp", name="sp"
                                    )
                                    nc.vector.tensor_add(
                                        sp,
                                        e_pairs[p][:, 0, :],
                                        e_pairs[p][:, 1, :],
                                    )
                                    nc.vector.tensor_add(acc, acc, sp)
                                if p + 2 < npr:
                                    emit_pair(p + 2)
                                emit_s3_group()
                                if bi == b - 1 and h == 1:
                                    emit_s3_group()
                            # partition-broadcast the denominator with one
                            # ones-matmul, then one Newton step for 1/den
                            dnb = psp.tile([1, qch], F32, tag="s3", name="dnb")
                            nc.tensor.matmul(
                                dnb, ones_sb[:, 0:1], acc, start=True, stop=True
                            )
                            # exact 1/den: copy the [1,qch] row out, bounce
                            # it through DRAM to a [128,4] layout, take the
                            # reciprocal on full lanes, bounce back as a
                            # 0-stride broadcast read. ~5us of DMA latency,
                            # fully hidden: attn_n[qc] is first consumed by
                            # the out-projection a chunk later.
                            drow = s2pool.tile([1, qch], F32, tag="drow", name="drow")
                            nc.vector.tensor_copy(drow, dnb)
                            nc.gpsimd.dma_start(
                                out=dnrow_d[h % 2, qc : qc + 1, :], in_=drow
                            )
                            rsm = s2pool.tile(
                                [128, qch // 128], F32, tag="rsm", name="rsm"
                            )
                            nc.gpsimd.dma_start(
                                out=rsm,
                                in_=dnrow_d[h % 2, qc, :].rearrange(
                                    "(p i) -> p i", p=128
                                ),
                            )
                            nc.vector.reciprocal(rsm, rsm)
                            nc.gpsimd.dma_start(
                                out=rcp_d[h % 2, qc, :].rearrange(
                                    "(p i) -> p i", p=128
                                ),
                                in_=rsm,
                            )
                            rcp = s2pool.tile(
                                [128, qch], F32, tag="rcp", name="rcp"
                            )
                            rsrc = rcp_d[h % 2, qc : qc + 1, :]
                            bcast = bass.AP(
                                tensor=rsrc.tensor,
                                offset=rsrc.offset,
                                ap=[[0, 128]] + [list(p) for p in rsrc.ap[1:]],
                            )
                            nc.gpsimd.dma_start(out=rcp, in_=bcast)
                            if qc < nqc - 1:
                                pending_mul.append((pv, rcp, qsl))
                            else:
                                nc.vector.tensor_mul(
                                    attn_n[:, h, qsl], pv, rcp
                                )
                            # out-projection chunk qc-1 becomes eligible one
                            # chunk after head 1 normalizes it, so the
                            # reciprocal's DMA-bounce latency stays hidden
                            if h == 1:
                                if bi < b - 1 and qc >= 1:
                                    s3_pending.extend(
                                        (attn_n, bi, qc - 1, do)
                                        for do in range(dt_)
                                    )
                                elif bi == b - 1:
                                    s3_pending.extend(
                                        (attn_n, bi, qc, do)
                                        for do in range(dt_)
                                    )
                    if h == 1 and bi < b - 1:
                        for cq in (nqc - 1,):
                            for do in range(dt_):
                                s3_pending.append((attn_n, bi, cq, do))
                # drain the final batch's remaining out-projection groups
                while s3_pending:
                    emit_s3_group()
            s2in.release()

    nc.finalize()
    return nc


_module_cache = {}


def _get_module(b, t, d_model, n_cores):
    key = (b, t, d_model, n_cores)
    if key not in _module_cache:
        _module_cache[key] = build_module(b, t, d_model, n_cores)
    return _module_cache[key]


def _host_tables(t):
    half = HEAD_DIM // 2
    theta = 1.0 / (
        np.float32(ROPE_BASE)
        ** (np.arange(half, dtype=np.float32) / np.float32(half))
    )
    freqs = np.arange(t, dtype=np.float32)[:, None] * theta[None, :]
    emb = np.concatenate([freqs, freqs], axis=-1)  # (t, 128)
    cosT = np.ascontiguousarray(np.cos(emb).T.astype(np.float32))
    sinT = np.sin(emb).T.astype(np.float32)
    rsinT = sinT.copy()
    rsinT[:half] = -sinT[:half]
    rsinT = np.ascontiguousarray(rsinT)
    return cosT, rsinT


def _run(x, Wq, Wk, Wv, Wo, trace=False):
    import ml_dtypes

    bf16 = ml_dtypes.bfloat16
    b_, t_, d_ = x.shape
    n_cores = (d_ // HEAD_DIM) // HPC
    nc = _get_module(b_, t_, d_, n_cores)

    xT = np.ascontiguousarray(x.reshape(b_ * t_, d_).T).astype(bf16)
    cosT, rsinT = _host_tables(t_)
    ident = np.eye(128, dtype=np.float32).astype(bf16)
    onesq = np.ones((128, 128), dtype=np.float32).astype(bf16)

    in_maps = []
    for c in range(n_cores):
        fs = slice(c * F_LOC, (c + 1) * F_LOC)
        in_maps.append(
            {
                "xT": xT,
                "wqT": np.ascontiguousarray(Wq[fs, :].T).astype(bf16),
                "wkT": np.ascontiguousarray(Wk[fs, :].T).astype(bf16),
                "wvT": np.ascontiguousarray(Wv[fs, :].T).astype(bf16),
                "woT": np.ascontiguousarray(Wo[:, fs].T).astype(bf16),
                "cosT": cosT,
                "rsinT": rsinT,
                "ident": ident,
                "onesq": onesq,
            }
        )
    res = run_bass_kernel_spmd(
        nc, in_maps, core_ids=list(range(n_cores)), trace=trace
    )
    acc = res.results[0]["outP"].astype(np.float32)
    for c in range(1, n_cores):
        acc += res.results[c]["outP"].astype(np.float32)
    out = np.ascontiguousarray(acc.T).reshape(b_, t_, d_)
    return out, res


def kernel(x, Wq, Wk, Wv, Wo):
    x = np.asarray(x, dtype=np.float32)
    Wq = np.asarray(Wq, dtype=np.float32)
    Wk = np.asarray(Wk, dtype=np.float32)
    Wv = np.asarray(Wv, dtype=np.float32)
    Wo = np.asarray(Wo, dtype=np.float32)
    out, _ = _run(x, Wq, Wk, Wv, Wo, trace=False)
    return out


if __name__ == "__main__":
    build_module()
    print("module built ok")


# revision 19
# speedup vs baseline: 1.0608x; 1.0608x over previous
"""Trainium2 Bass kernel: 16-head RoPE attention block (B=4, T=2048, D=2048).

Sharding: tensor-parallel over heads. Each of the 8 cores owns 2 heads
(a 256-wide slice of the q/k/v projection output features). Per core:

  stage 1: q/k/v projections in feature-major layout, all-bf16 matmuls
           (x and W arrive bf16: halves DMA + SBUF traffic at the same
           1-col/cycle PE rate), RoPE on the vector engine (bf16 out),
           v transposed to token-major via the PE; results staged in
           DRAM per (head, batch). Chunk 0 is quarter-reordered (q/k
           groups track the x quarters as they land; v groups follow)
           with the loads spread over the three DMA rings.
  stage 2: per (batch, head): scores computed TRANSPOSED (S^T[k,q] =
           kTile^T @ qT, bf16) in PAIRS of k-tiles sharing one two-bank
           PSUM tile so each scalar-engine EXP covers 1024 columns (the
           352-cycle ACT overhead amortizes and the scalar engine stays
           below the PE; only the exp table set is ever loaded). The
           softmax denominator accumulates as bf16 pair-sums + a running
           bf16 acc on the vector engine, is collapsed to a [1,qch] row
           by a ones-column matmul, bounced through DRAM into a [128,4]
           layout for a full-lane DVE reciprocal, and broadcast back
           with a 0-stride DMA read. The ~10us of bounce latency is
           hidden: out-projection work referencing a chunk is deferred
           by one query chunk.
  stage 3: out-projection (bf16 x bf16) matmuls INTERLEAVED into the
           attention loop one query-chunk later, filling the PE's
           exp-wait gaps; evacuated to bf16 (3:1 vector/scalar split)
           and DMAed out on the sync ring.

Host sums the 8 bf16 partial outputs (the "all-reduce") in f32 and
un-transposes. PSUM budget: score pairs 2x2 banks + pv 2 + s3/dnrow 2
= 8 banks. Stage-2 input q/k/v tiles live in a right-side SBUF pool
that outlives stage 1, so batch 0 prefetches with no WAR stall at the
stage boundary. Measured: 902.7us, rel err 5.9e-3 (vs 1067.1us
baseline).
"""

import math

import numpy as np

import concourse.bacc as bacc
import concourse.bass as bass
import concourse.mybir as mybir
import concourse.tile as tile
from concourse.bass_utils import run_bass_kernel_spmd

F32 = mybir.dt.float32
F32R = mybir.dt.float32r
BF16 = mybir.dt.bfloat16
EXP = mybir.ActivationFunctionType.Exp
LN = mybir.ActivationFunctionType.Ln

# Problem shape (hardcoded; the harness calls kernel() with exactly these).
B = 4
T = 2048
D_MODEL = 2048
HEAD_DIM = 128
N_CORES = 8
ROPE_BASE = 10000.0

HPC = 2                      # heads per core
F_LOC = HPC * HEAD_DIM       # 256 local projection features per core
BT = B * T
TCH = 512                    # token chunk width (stages 1/3)
QCH = 512                    # query chunk width (stage 2)
SCALE = 1.0 / math.sqrt(HEAD_DIM)


def build_module(b=B, t=T, d_model=D_MODEL, n_cores=N_CORES):
    """Build the per-core Bass module. All cores run the same program on
    different data (pure SPMD, no collectives)."""
    bt = b * t
    dt_ = d_model // 128
    kt = t // 128
    npr = kt // 2            # k-tile pairs per query chunk
    tch = min(TCH, bt)
    qch = min(QCH, t)
    ntch = bt // tch
    nqc = t // qch
    cpb = t // tch           # stage-1/3 token chunks per batch
    qd = dt_ // 4            # d-tile quarter for the startup loads

    nc = bacc.Bacc(None, target_bir_lowering=False)

    xT = nc.dram_tensor("xT", [d_model, bt], BF16, kind="ExternalInput")
    wqT = nc.dram_tensor("wqT", [d_model, F_LOC], BF16, kind="ExternalInput")
    wkT = nc.dram_tensor("wkT", [d_model, F_LOC], BF16, kind="ExternalInput")
    wvT = nc.dram_tensor("wvT", [d_model, F_LOC], BF16, kind="ExternalInput")
    woT = nc.dram_tensor("woT", [F_LOC, d_model], BF16, kind="ExternalInput")
    cosT = nc.dram_tensor("cosT", [HEAD_DIM, t], F32, kind="ExternalInput")
    rsinT = nc.dram_tensor("rsinT", [HEAD_DIM, t], F32, kind="ExternalInput")
    ident = nc.dram_tensor("ident", [128, 128], BF16, kind="ExternalInput")
    onesq = nc.dram_tensor("onesq", [128, 128], BF16, kind="ExternalInput")
    outP = nc.dram_tensor("outP", [d_model, bt], BF16, kind="ExternalOutput")

    with tile.TileContext(nc) as tc:
        with (
            tc.tile_pool(name="const", bufs=1) as constp,
            tc.tile_pool(name="dram", bufs=1, space="DRAM") as dram,
            tc.tile_pool(name="psp", bufs=2, space="PSUM") as psp,
        ):
            # constants: tiles here, DMAs emitted inside stage 1 so the
            # per-ring issue order puts the matmul-gating loads first
            cos_sb = constp.tile([128, t], F32)
            rsin_sb = constp.tile([128, t], F32)
            id_sb = constp.tile([128, 128], BF16)
            ones_sb = constp.tile([128, 128], BF16)

            # DRAM scratch, per (head, batch): cross-stage deps stay
            # batch-granular so the stages pipeline
            q_scr = [
                [dram.tile([128, t], BF16, name=f"qs{h}_{bi}", tag=f"qs{h}_{bi}") for bi in range(b)]
                for h in range(HPC)
            ]
            k_scr = [
                [dram.tile([128, t], BF16, name=f"ks{h}_{bi}", tag=f"ks{h}_{bi}") for bi in range(b)]
                for h in range(HPC)
            ]
            v_scr = [
                [dram.tile([kt, 128, 128], BF16, name=f"vs{h}_{bi}", tag=f"vs{h}_{bi}") for bi in range(b)]
                for h in range(HPC)
            ]

            prefetched = {}
            # per-qc scratch rows for the DMA-reshaped reciprocal
            dnrow_d = dram.tile([2, t // 128 // 4, 512], F32, name="dnrow_d")
            rcp_d = dram.tile([2, t // 128 // 4, 512], F32, name="rcp_d")

            # ================= stage 1: projections + rope + v^T =========
            # manual pool lifetimes: the stage-1 pools take the low SBUF
            # addresses; s2in is allocated above them and outlives stage 1
            # (the batch-0 q/k/v prefetch needs no WAR wait on stage-1
            # buffers at the stage boundary)
            wpool = tc.alloc_tile_pool(name="s1w", bufs=1)
            xpool = tc.alloc_tile_pool(name="s1x", bufs=3)
            tpool = tc.alloc_tile_pool(name="s1t", bufs=4)
            s2in = tc.alloc_tile_pool(name="s2in", bufs=2, side="right")
            if True:
                w_sbs = []
                wsrcs = []
                for wten, wname in ((wqT, "wq"), (wkT, "wk"), (wvT, "wv")):
                    wsb = wpool.tile([128, dt_, F_LOC], BF16, tag=wname)
                    w_sbs.append(wsb)
                    wsrcs.append(
                        wten[:, :].rearrange("(dt p) f -> p dt f", p=128)
                    )

                # ---- startup loads over the three DMA rings ----
                x0 = xpool.tile([128, dt_, tch], BF16, tag="x")
                x0src = xT[:, 0:tch].rearrange("(dt p) tt -> p dt tt", p=128)
                # sync ring: the four x chunk-0 quarters (chunks 2+ follow)
                for qtr in range(4):
                    dsl = slice(qtr * qd, (qtr + 1) * qd)
                    nc.sync.dma_start(out=x0[:, dsl, :], in_=x0src[:, dsl, :])
                # scalar ring: wq quarters (x chunk 1 + wo follow)
                for qtr in range(4):
                    dsl = slice(qtr * qd, (qtr + 1) * qd)
                    nc.scalar.dma_start(
                        out=w_sbs[0][:, dsl, :], in_=wsrcs[0][:, dsl, :]
                    )
                # gpsimd ring: wk quarters, wv quarters, then constants
                for wi in (1, 2):
                    for qtr in range(4):
                        dsl = slice(qtr * qd, (qtr + 1) * qd)
                        nc.gpsimd.dma_start(
                            out=w_sbs[wi][:, dsl, :], in_=wsrcs[wi][:, dsl, :]
                        )
                nc.gpsimd.dma_start(out=cos_sb, in_=cosT[:, :])
                nc.gpsimd.dma_start(out=rsin_sb, in_=rsinT[:, :])
                nc.gpsimd.dma_start(out=id_sb, in_=ident[:, :])
                nc.gpsimd.dma_start(out=ones_sb, in_=onesq[:, :])

                def evac_qk(pi, ft, ps, bi, off):
                    """RoPE + bf16 store for one q/k psum group."""
                    lsl = slice(off, off + tch)
                    ro = tpool.tile([128, tch], F32, tag="ro")
                    nc.vector.tensor_mul(ro, ps, cos_sb[:, lsl])
                    rt = tpool.tile([128, tch], F32, tag="rt")
                    nc.vector.tensor_mul(rt[0:64], ps[64:128], rsin_sb[0:64, lsl])
                    nc.vector.tensor_mul(rt[64:128], ps[0:64], rsin_sb[64:128, lsl])
                    rs = tpool.tile([128, tch], BF16, tag="rs")
                    nc.vector.tensor_add(rs, ro, rt)
                    scr = q_scr if pi == 0 else k_scr
                    nc.gpsimd.dma_start(out=scr[ft][bi][:, lsl], in_=rs)

                def evac_v(ft, ps, bi, tch_i):
                    """bf16 copy + PE transpose + store for one v group."""
                    vsb = tpool.tile([128, tch], BF16, tag="vs")
                    nc.scalar.copy(vsb, ps)
                    for j in range(tch // 128):
                        pst = psp.tile([128, 128], BF16, tag="s3", name="pst")
                        nc.tensor.transpose(
                            pst, vsb[:, j * 128 : (j + 1) * 128], id_sb
                        )
                        vt = tpool.tile([128, 128], BF16, tag="vt")
                        nc.vector.tensor_copy(vt, pst)
                        nc.gpsimd.dma_start(
                            out=v_scr[ft][bi][
                                (tch_i % cpb) * (tch // 128) + j, :, :
                            ],
                            in_=vt,
                        )

                # ---- chunk 0: q/k groups follow the x quarters; v after ----
                qk_pairs = []
                for pi in range(2):
                    scp = psp.tile([128, 2, tch], F32, tag="sc", name=f"c0qk{pi}")
                    qk_pairs.append(scp)
                v_ps = []
                for ft in range(HPC):
                    vps = psp.tile([128, tch], F32, tag="pv", name=f"c0v{ft}")
                    v_ps.append(vps)
                for dq in range(4):
                    for pi in range(2):
                        for ft in range(HPC):
                            fsl = slice(ft * 128, (ft + 1) * 128)
                            for di in range(dq * qd, (dq + 1) * qd):
                                nc.tensor.matmul(
                                    qk_pairs[pi][:, ft, :],
                                    w_sbs[pi][:, di, fsl],
                                    x0[:, di, :],
                                    start=(di == 0),
                                    stop=(di == dt_ - 1),
                                )
                for ft in range(HPC):
                    fsl = slice(ft * 128, (ft + 1) * 128)
                    for di in range(dt_):
                        nc.tensor.matmul(
                            v_ps[ft],
                            w_sbs[2][:, di, fsl],
                            x0[:, di, :],
                            start=(di == 0),
                            stop=(di == dt_ - 1),
                        )
                for pi in range(2):
                    for ft in range(HPC):
                        evac_qk(pi, ft, qk_pairs[pi][:, ft, :], 0, 0)
                for ft in range(HPC):
                    evac_v(ft, v_ps[ft], 0, 0)

                # ---- chunks 1..ntch-1 ----
                for tch_i in range(1, ntch):
                    bi = tch_i // cpb
                    off = (tch_i % cpb) * tch
                    tsl = slice(tch_i * tch, (tch_i + 1) * tch)
                    x_sb = xpool.tile([128, dt_, tch], BF16, tag="x")
                    xsrc = xT[:, tsl].rearrange("(dt p) tt -> p dt tt", p=128)
                    # chunk 1 rides the scalar ring (free after wq) so it
                    # lands before chunk 0's compute finishes
                    ring = nc.scalar if tch_i == 1 else nc.sync
                    ring.dma_start(out=x_sb, in_=xsrc)
                    for pi in range(2):
                        scp = psp.tile(
                            [128, 2, tch], F32, tag="sc", name=f"qk{tch_i}_{pi}"
                        )
                        for ft in range(HPC):
                            fsl = slice(ft * 128, (ft + 1) * 128)
                            for di in range(dt_):
                                nc.tensor.matmul(
                                    scp[:, ft, :],
                                    w_sbs[pi][:, di, fsl],
                                    x_sb[:, di, :],
                                    start=(di == 0),
                                    stop=(di == dt_ - 1),
                                )
                        for ft in range(HPC):
                            evac_qk(pi, ft, scp[:, ft, :], bi, off)
                    for ft in range(HPC):
                        fsl = slice(ft * 128, (ft + 1) * 128)
                        vps = psp.tile(
                            [128, tch], F32, tag="pv", name=f"v{tch_i}_{ft}"
                        )
                        for di in range(dt_):
                            nc.tensor.matmul(
                                vps,
                                w_sbs[2][:, di, fsl],
                                x_sb[:, di, :],
                                start=(di == 0),
                                stop=(di == dt_ - 1),
                            )
                        evac_v(ft, vps, bi, tch_i)

                    # prefetch batch-0 q/k/v into the pre-opened s2in pool
                    # (disjoint SBUF: no WAR wait on stage-1 buffers at the
                    # stage boundary)
                    if tch_i == cpb - 1:
                        for h in range(HPC):
                            pq = s2in.tile([128, t], BF16, tag="q", name=f"pq{h}")
                            nc.sync.dma_start(out=pq, in_=q_scr[h][0][:, :])
                            pk = s2in.tile([128, t], BF16, tag="k", name=f"pk{h}")
                            nc.sync.dma_start(out=pk, in_=k_scr[h][0][:, :])
                            pv_ = s2in.tile(
                                [128, kt, 128], BF16, tag="v", name=f"pv{h}"
                            )
                            nc.sync.dma_start(
                                out=pv_,
                                in_=v_scr[h][0][:, :, :].rearrange(
                                    "tt p dh -> p tt dh"
                                ),
                            )
                            prefetched[h] = (pq, pk, pv_)

            tpool.release()
            xpool.release()
            wpool.release()

            # ======== stage 2+3: attention + interleaved out-projection ====
            with (
                tc.tile_pool(name="s2", bufs=2) as s2pool,
                tc.tile_pool(name="s2e", bufs=5) as epool,
                tc.tile_pool(name="s3w", bufs=1) as wopool,
                tc.tile_pool(name="s3o", bufs=6) as s3pool,
            ):
                wo_sb = wopool.tile([128, HPC, d_model], BF16, tag="wo")
                nc.scalar.dma_start(
                    out=wo_sb,
                    in_=woT[:, :].rearrange("(ft p) d -> p ft d", p=128),
                )

                # pending out-projection groups: (attn_tile, bi, c4, do)
                s3_pending = []
                s3_count = [0]

                def emit_s3_group():
                    if not s3_pending:
                        return
                    attn_src, bi_src, c4, do = s3_pending.pop(0)
                    off = c4 * tch
                    gsl = slice(bi_src * t + off, bi_src * t + off + tch)
                    ps = psp.tile([128, tch], F32, tag="s3", name="s3ps")
                    for ft in range(HPC):
                        nc.tensor.matmul(
                            ps,
                            wo_sb[:, ft, do * 128 : (do + 1) * 128],
                            attn_src[:, ft, off : off + tch],
                            start=(ft == 0),
                            stop=(ft == HPC - 1),
                        )
                    osb = s3pool.tile([128, tch], BF16, tag="o")
                    if s3_count[0] % 4 == 3:
                        nc.scalar.copy(osb, ps)
                    else:
                        nc.vector.tensor_copy(osb, ps)
                    ring = nc.sync if s3_count[0] % 2 == 0 else nc.gpsimd
                    s3_count[0] += 1
                    ring.dma_start(
                        out=outP[do * 128 : (do + 1) * 128, gsl], in_=osb
                    )

                for bi in range(b):
                    attn_n = s2pool.tile([128, HPC, t], BF16, tag="an")
                    for h in range(HPC):
                        if bi == 0:
                            q_sb, k_sb, v_sb = prefetched[h]
                        else:
                            q_sb = s2in.tile([128, t], BF16, tag="q")
                            nc.sync.dma_start(out=q_sb, in_=q_scr[h][bi][:, :])
                            k_sb = s2in.tile([128, t], BF16, tag="k")
                            nc.sync.dma_start(out=k_sb, in_=k_scr[h][bi][:, :])
                            v_sb = s2in.tile([128, kt, 128], BF16, tag="v")
                            nc.sync.dma_start(
                                out=v_sb,
                                in_=v_scr[h][bi][:, :, :].rearrange(
                                    "tt p dh -> p tt dh"
                                ),
                            )
                        pending_mul = []
                        for qc in range(nqc):
                            qsl = slice(qc * qch, (qc + 1) * qch)
                            e_pairs = [None] * npr

                            def emit_pair(p):
                                sps = psp.tile(
                                    [128, 2, qch], F32, tag="sc", name="sps"
                                )
                                for j in range(2):
                                    kti = 2 * p + j
                                    nc.tensor.matmul(
                                        sps[:, j, :],
                                        k_sb[:, kti * 128 : (kti + 1) * 128],
                                        q_sb[:, qsl],
                                        start=True,
                                        stop=True,
                                    )
                                e_p = epool.tile(
                                    [128, 2, qch], BF16, tag="E", name="e_p"
                                )
                                # one EXP over both k-tiles (1024 cols):
                                # amortizes the 352-cycle ACT overhead
                                nc.scalar.activation(e_p, sps, EXP, scale=SCALE)
                                e_pairs[p] = e_p

                            emit_pair(0)
                            emit_pair(1)
                            while pending_mul:
                                m_pv, m_rcp, m_sl = pending_mul.pop(0)
                                nc.vector.tensor_mul(
                                    attn_n[:, h, m_sl], m_pv, m_rcp
                                )
                            pv = psp.tile([128, qch], F32, tag="pv", name="pv")
                            acc = s2pool.tile(
                                [128, qch], BF16, tag="acc", name="acc"
                            )
                            for p in range(npr):
                                for j in range(2):
                                    nc.tensor.matmul(
                                        pv,
                                        v_sb[:, 2 * p + j, :],
                                        e_pairs[p][:, j, :],
                                        start=(p == 0 and j == 0),
                                        stop=(p == npr - 1 and j == 1),
                                    )
                                # denominator: bf16 pair-collapse (gpsimd)
                                # + running bf16 acc (vector)
                                if p == 0:
                                    nc.vector.tensor_add(
                                        acc,
                                        e_pairs[0][:, 0, :],
                                        e_pairs[0][:, 1, :],
                                    )
                                else:
                                    sp = s2pool.tile(
                                        [128, qch], BF16, tag="sp", name="sp"
                                    )
                                    nc.vector.tensor_add(
                                        sp,
                                        e_pairs[p][:, 0, :],
                                        e_pairs[p][:, 1, :],
                                    )
                                    nc.vector.tensor_add(acc, acc, sp)
                                if p + 2 < npr:
                                    emit_pair(p + 2)
                                emit_s3_group()
                            # partition-broadcast the denominator with one
                            # ones-matmul, then one Newton step for 1/den
                            dnb = psp.tile([1, qch], F32, tag="s3", name="dnb")
                            nc.tensor.matmul(
                                dnb, ones_sb[:, 0:1], acc, start=True, stop=True
                            )
                            # exact 1/den: copy the [1,qch] row out, bounce
                            # it through DRAM to a [128,4] layout, take the
                            # reciprocal on full lanes, bounce back as a
                            # 0-stride broadcast read. ~5us of DMA latency,
                            # fully hidden: attn_n[qc] is first consumed by
                            # the out-projection a chunk later.
                            drow = s2pool.tile([1, qch], F32, tag="drow", name="drow")
                            nc.vector.tensor_copy(drow, dnb)
                            nc.gpsimd.dma_start(
                                out=dnrow_d[h % 2, qc : qc + 1, :], in_=drow
                            )
                            rsm = s2pool.tile(
                                [128, qch // 128], F32, tag="rsm", name="rsm"
                            )
                            nc.gpsimd.dma_start(
                                out=rsm,
                                in_=dnrow_d[h % 2, qc, :].rearrange(
                                    "(p i) -> p i", p=128
                                ),
                            )
                            nc.vector.reciprocal(rsm, rsm)
                            nc.gpsimd.dma_start(
                                out=rcp_d[h % 2, qc, :].rearrange(
                                    "(p i) -> p i", p=128
                                ),
                                in_=rsm,
                            )
                            rcp = s2pool.tile(
                                [128, qch], F32, tag="rcp", name="rcp"
                            )
                            rsrc = rcp_d[h % 2, qc : qc + 1, :]
                            bcast = bass.AP(
                                tensor=rsrc.tensor,
                                offset=rsrc.offset,
                                ap=[[0, 128]] + [list(p) for p in rsrc.ap[1:]],
                            )
                            nc.gpsimd.dma_start(out=rcp, in_=bcast)
                            if qc < nqc - 1:
                                pending_mul.append((pv, rcp, qsl))
                            else:
                                nc.vector.tensor_mul(
                                    attn_n[:, h, qsl], pv, rcp
                                )
                            # out-projection chunk qc-1 becomes eligible one
                            # chunk after head 1 normalizes it, so the
                            # reciprocal's DMA-bounce latency stays hidden
                            if h == 1 and qc >= 1:
                                for do in range(dt_):
                                    s3_pending.append((attn_n, bi, qc - 1, do))
                    if h == 1:
                        for cq in (nqc - 1,):
                            for do in range(dt_):
                                s3_pending.append((attn_n, bi, cq, do))
                # drain the final batch's remaining out-projection groups
                while s3_pending:
                    emit_s3_group()
            s2in.release()

    nc.finalize()
    return nc


_module_cache = {}


def _get_module(b, t, d_model, n_cores):
    key = (b, t, d_model, n_cores)
    if key not in _module_cache:
        _module_cache[key] = build_module(b, t, d_model, n_cores)
    return _module_cache[key]


def _host_tables(t):
    half = HEAD_DIM // 2
    theta = 1.0 / (
        np.float32(ROPE_BASE)
        ** (np.arange(half, dtype=np.float32) / np.float32(half))
    )
    freqs = np.arange(t, dtype=np.float32)[:, None] * theta[None, :]
    emb = np.concatenate([freqs, freqs], axis=-1)  # (t, 128)
    cosT = np.ascontiguousarray(np.cos(emb).T.astype(np.float32))
    sinT = np.sin(emb).T.astype(np.float32)
    rsinT = sinT.copy()
    rsinT[:half] = -sinT[:half]
    rsinT = np.ascontiguousarray(rsinT)
    return cosT, rsinT


def _run(x, Wq, Wk, Wv, Wo, trace=False):
    import ml_dtypes

    bf16 = ml_dtypes.bfloat16
    b_, t_, d_ = x.shape
    n_cores = (d_ // HEAD_DIM) // HPC
    nc = _get_module(b_, t_, d_, n_cores)

    xT = np.ascontiguousarray(x.reshape(b_ * t_, d_).T).astype(bf16)
    cosT, rsinT = _host_tables(t_)
    ident = np.eye(128, dtype=np.float32).astype(bf16)
    onesq = np.ones((128, 128), dtype=np.float32).astype(bf16)

    in_maps = []
    for c in range(n_cores):
        fs = slice(c * F_LOC, (c + 1) * F_LOC)
        in_maps.append(
            {
                "xT": xT,
                "wqT": np.ascontiguousarray(Wq[fs, :].T).astype(bf16),
                "wkT": np.ascontiguousarray(Wk[fs, :].T).astype(bf16),
                "wvT": np.ascontiguousarray(Wv[fs, :].T).astype(bf16),
                "woT": np.ascontiguousarray(Wo[:, fs].T).astype(bf16),
                "cosT": cosT,
                "rsinT": rsinT,
                "ident": ident,
                "onesq": onesq,
            }
        )
    res = run_bass_kernel_spmd(
        nc, in_maps, core_ids=list(range(n_cores)), trace=trace
    )
    acc = res.results[0]["outP"].astype(np.float32)
    for c in range(1, n_cores):
        acc += res.results[c]["outP"].astype(np.float32)
    out = np.ascontiguousarray(acc.T).reshape(b_, t_, d_)
    return out, res


def kernel(x, Wq, Wk, Wv, Wo):
    x = np.asarray(x, dtype=np.float32)
    Wq = np.asarray(Wq, dtype=np.float32)
    Wk = np.asarray(Wk, dtype=np.float32)
    Wv = np.asarray(Wv, dtype=np.float32)
    Wo = np.asarray(Wo, dtype=np.float32)
    out, _ = _run(x, Wq, Wk, Wv, Wo, trace=False)
    return out


if __name__ == "__main__":
    build_module()
    print("module built ok")


# revision 21
# speedup vs baseline: 1.0751x; 1.0135x over previous
"""Trainium2 Bass kernel: 16-head RoPE attention block (B=4, T=2048, D=2048).

Sharding: tensor-parallel over heads. Each of the 8 cores owns 2 heads
(a 256-wide slice of the q/k/v projection output features). Per core:

  stage 1: q/k/v projections in feature-major layout, all-bf16 matmuls
           (x and W arrive bf16: halves DMA + SBUF traffic at the same
           1-col/cycle PE rate), RoPE on the vector engine (bf16 out),
           v transposed to token-major via the PE; results staged in
           DRAM per (head, batch). Chunk 0 is quarter-reordered (q/k
           groups track the x quarters as they land; v groups follow)
           with the loads spread over the three DMA rings.
  stage 2: per (batch, head): scores computed TRANSPOSED (S^T[k,q] =
           kTile^T @ qT, bf16) in PAIRS of k-tiles sharing one two-bank
           PSUM tile so each scalar-engine EXP covers 1024 columns (the
           352-cycle ACT overhead amortizes and the scalar engine stays
           below the PE; only the exp table set is ever loaded). The
           softmax denominator accumulates as bf16 pair-sums + a running
           bf16 acc on the vector engine, is collapsed to a [1,qch] row
           by a ones-column matmul, bounced through DRAM into a [128,4]
           layout for a full-lane DVE reciprocal, and broadcast back
           with a 0-stride DMA read. The ~10us of bounce latency is
           hidden: out-projection work referencing a chunk is deferred
           by one query chunk.
  stage 3: out-projection (bf16 x bf16) matmuls INTERLEAVED into the
           attention loop one query-chunk later, filling the PE's
           exp-wait gaps; evacuated to bf16 (3:1 vector/scalar split)
           and DMAed out on the sync ring.

Host sums the 8 bf16 partial outputs (the "all-reduce") in f32 and
un-transposes. PSUM budget: score pairs 2x2 banks + pv 2 + s3/dnrow 2
= 8 banks. Stage-2 input q/k/v tiles live in a right-side SBUF pool
that outlives stage 1, so batch 0 prefetches with no WAR stall at the
stage boundary. The attn_n normalize-multiply is itself deferred into
the next chunk's pair loop so the in-order DVE queue never blocks on
the bounce DMA. Measured: 872.9us, rel err 5.9e-3 (vs 1067.1us
baseline).
"""

import math

import numpy as np

import concourse.bacc as bacc
import concourse.bass as bass
import concourse.mybir as mybir
import concourse.tile as tile
from concourse.bass_utils import run_bass_kernel_spmd

F32 = mybir.dt.float32
F32R = mybir.dt.float32r
BF16 = mybir.dt.bfloat16
EXP = mybir.ActivationFunctionType.Exp
LN = mybir.ActivationFunctionType.Ln

# Problem shape (hardcoded; the harness calls kernel() with exactly these).
B = 4
T = 2048
D_MODEL = 2048
HEAD_DIM = 128
N_CORES = 8
ROPE_BASE = 10000.0

HPC = 2                      # heads per core
F_LOC = HPC * HEAD_DIM       # 256 local projection features per core
BT = B * T
TCH = 512                    # token chunk width (stages 1/3)
QCH = 512                    # query chunk width (stage 2)
SCALE = 1.0 / math.sqrt(HEAD_DIM)


def build_module(b=B, t=T, d_model=D_MODEL, n_cores=N_CORES):
    """Build the per-core Bass module. All cores run the same program on
    different data (pure SPMD, no collectives)."""
    bt = b * t
    dt_ = d_model // 128
    kt = t // 128
    npr = kt // 2            # k-tile pairs per query chunk
    tch = min(TCH, bt)
    qch = min(QCH, t)
    ntch = bt // tch
    nqc = t // qch
    cpb = t // tch           # stage-1/3 token chunks per batch
    qd = dt_ // 4            # d-tile quarter for the startup loads

    nc = bacc.Bacc(None, target_bir_lowering=False)

    xT = nc.dram_tensor("xT", [d_model, bt], BF16, kind="ExternalInput")
    wqT = nc.dram_tensor("wqT", [d_model, F_LOC], BF16, kind="ExternalInput")
    wkT = nc.dram_tensor("wkT", [d_model, F_LOC], BF16, kind="ExternalInput")
    wvT = nc.dram_tensor("wvT", [d_model, F_LOC], BF16, kind="ExternalInput")
    woT = nc.dram_tensor("woT", [F_LOC, d_model], BF16, kind="ExternalInput")
    cosT = nc.dram_tensor("cosT", [HEAD_DIM, t], F32, kind="ExternalInput")
    rsinT = nc.dram_tensor("rsinT", [HEAD_DIM, t], F32, kind="ExternalInput")
    ident = nc.dram_tensor("ident", [128, 128], BF16, kind="ExternalInput")
    onesq = nc.dram_tensor("onesq", [128, 128], BF16, kind="ExternalInput")
    outP = nc.dram_tensor("outP", [d_model, bt], BF16, kind="ExternalOutput")

    with tile.TileContext(nc) as tc:
        with (
            tc.tile_pool(name="const", bufs=1) as constp,
            tc.tile_pool(name="dram", bufs=1, space="DRAM") as dram,
            tc.tile_pool(name="psp", bufs=2, space="PSUM") as psp,
        ):
            # constants: tiles here, DMAs emitted inside stage 1 so the
            # per-ring issue order puts the matmul-gating loads first
            cos_sb = constp.tile([128, t], F32)
            rsin_sb = constp.tile([128, t], F32)
            id_sb = constp.tile([128, 128], BF16)
            ones_sb = constp.tile([128, 128], BF16)

            # DRAM scratch, per (head, batch): cross-stage deps stay
            # batch-granular so the stages pipeline
            q_scr = [
                [dram.tile([128, t], BF16, name=f"qs{h}_{bi}", tag=f"qs{h}_{bi}") for bi in range(b)]
                for h in range(HPC)
            ]
            k_scr = [
                [dram.tile([128, t], BF16, name=f"ks{h}_{bi}", tag=f"ks{h}_{bi}") for bi in range(b)]
                for h in range(HPC)
            ]
            v_scr = [
                [dram.tile([kt, 128, 128], BF16, name=f"vs{h}_{bi}", tag=f"vs{h}_{bi}") for bi in range(b)]
                for h in range(HPC)
            ]

            prefetched = {}
            # per-qc scratch rows for the DMA-reshaped reciprocal
            dnrow_d = dram.tile([2, t // 128 // 4, 512], F32, name="dnrow_d")
            rcp_d = dram.tile([2, t // 128 // 4, 512], F32, name="rcp_d")

            # ================= stage 1: projections + rope + v^T =========
            # manual pool lifetimes: the stage-1 pools take the low SBUF
            # addresses; s2in is allocated above them and outlives stage 1
            # (the batch-0 q/k/v prefetch needs no WAR wait on stage-1
            # buffers at the stage boundary)
            wpool = tc.alloc_tile_pool(name="s1w", bufs=1)
            xpool = tc.alloc_tile_pool(name="s1x", bufs=2)
            tpool = tc.alloc_tile_pool(name="s1t", bufs=4)
            s2in = tc.alloc_tile_pool(name="s2in", bufs=3, side="right")
            if True:
                w_sbs = []
                wsrcs = []
                for wten, wname in ((wqT, "wq"), (wkT, "wk"), (wvT, "wv")):
                    wsb = wpool.tile([128, dt_, F_LOC], BF16, tag=wname)
                    w_sbs.append(wsb)
                    wsrcs.append(
                        wten[:, :].rearrange("(dt p) f -> p dt f", p=128)
                    )

                # ---- startup loads over the three DMA rings ----
                x0 = xpool.tile([128, dt_, tch], BF16, tag="x")
                x0src = xT[:, 0:tch].rearrange("(dt p) tt -> p dt tt", p=128)
                # sync ring: the four x chunk-0 quarters (chunks 2+ follow)
                for qtr in range(4):
                    dsl = slice(qtr * qd, (qtr + 1) * qd)
                    nc.sync.dma_start(out=x0[:, dsl, :], in_=x0src[:, dsl, :])
                # scalar ring: wq quarters (x chunk 1 + wo follow)
                for qtr in range(4):
                    dsl = slice(qtr * qd, (qtr + 1) * qd)
                    nc.scalar.dma_start(
                        out=w_sbs[0][:, dsl, :], in_=wsrcs[0][:, dsl, :]
                    )
                # gpsimd ring: wk halves, wv halves, then constants
                for wi in (1, 2):
                    for hf in range(2):
                        dsl = slice(hf * 2 * qd, (hf + 1) * 2 * qd)
                        nc.gpsimd.dma_start(
                            out=w_sbs[wi][:, dsl, :], in_=wsrcs[wi][:, dsl, :]
                        )
                nc.gpsimd.dma_start(out=cos_sb, in_=cosT[:, :])
                nc.gpsimd.dma_start(out=rsin_sb, in_=rsinT[:, :])
                nc.gpsimd.dma_start(out=id_sb, in_=ident[:, :])
                nc.gpsimd.dma_start(out=ones_sb, in_=onesq[:, :])

                def evac_qk(pi, ft, ps, bi, off):
                    """RoPE + bf16 store for one q/k psum group."""
                    lsl = slice(off, off + tch)
                    ro = tpool.tile([128, tch], F32, tag="ro")
                    nc.vector.tensor_mul(ro, ps, cos_sb[:, lsl])
                    rt = tpool.tile([128, tch], F32, tag="rt")
                    nc.vector.tensor_mul(rt[0:64], ps[64:128], rsin_sb[0:64, lsl])
                    nc.vector.tensor_mul(rt[64:128], ps[0:64], rsin_sb[64:128, lsl])
                    rs = tpool.tile([128, tch], BF16, tag="rs")
                    nc.vector.tensor_add(rs, ro, rt)
                    scr = q_scr if pi == 0 else k_scr
                    nc.gpsimd.dma_start(out=scr[ft][bi][:, lsl], in_=rs)

                def evac_v(ft, ps, bi, tch_i):
                    """bf16 copy + PE transpose + store for one v group."""
                    vsb = tpool.tile([128, tch], BF16, tag="vs")
                    nc.scalar.copy(vsb, ps)
                    for j in range(tch // 128):
                        pst = psp.tile([128, 128], BF16, tag="s3", name="pst")
                        nc.tensor.transpose(
                            pst, vsb[:, j * 128 : (j + 1) * 128], id_sb
                        )
                        vt = tpool.tile([128, 128], BF16, tag="vt")
                        nc.vector.tensor_copy(vt, pst)
                        nc.gpsimd.dma_start(
                            out=v_scr[ft][bi][
                                (tch_i % cpb) * (tch // 128) + j, :, :
                            ],
                            in_=vt,
                        )

                # ---- chunk 0: q/k groups follow the x quarters; v after ----
                qk_pairs = []
                for pi in range(2):
                    scp = psp.tile([128, 2, tch], F32, tag="sc", name=f"c0qk{pi}")
                    qk_pairs.append(scp)
                v_ps = []
                for ft in range(HPC):
                    vps = psp.tile([128, tch], F32, tag="pv", name=f"c0v{ft}")
                    v_ps.append(vps)
                for dq in range(4):
                    for pi in range(2):
                        for ft in range(HPC):
                            fsl = slice(ft * 128, (ft + 1) * 128)
                            for di in range(dq * qd, (dq + 1) * qd):
                                nc.tensor.matmul(
                                    qk_pairs[pi][:, ft, :],
                                    w_sbs[pi][:, di, fsl],
                                    x0[:, di, :],
                                    start=(di == 0),
                                    stop=(di == dt_ - 1),
                                )
                for ft in range(HPC):
                    fsl = slice(ft * 128, (ft + 1) * 128)
                    for di in range(dt_):
                        nc.tensor.matmul(
                            v_ps[ft],
                            w_sbs[2][:, di, fsl],
                            x0[:, di, :],
                            start=(di == 0),
                            stop=(di == dt_ - 1),
                        )
                for pi in range(2):
                    for ft in range(HPC):
                        evac_qk(pi, ft, qk_pairs[pi][:, ft, :], 0, 0)
                for ft in range(HPC):
                    evac_v(ft, v_ps[ft], 0, 0)

                # ---- chunks 1..ntch-1 ----
                for tch_i in range(1, ntch):
                    bi = tch_i // cpb
                    off = (tch_i % cpb) * tch
                    tsl = slice(tch_i * tch, (tch_i + 1) * tch)
                    x_sb = xpool.tile([128, dt_, tch], BF16, tag="x")
                    xsrc = xT[:, tsl].rearrange("(dt p) tt -> p dt tt", p=128)
                    # chunk 1 rides the scalar ring (free after wq) so it
                    # lands before chunk 0's compute finishes
                    ring = nc.scalar if tch_i == 1 else nc.sync
                    ring.dma_start(out=x_sb, in_=xsrc)
                    for pi in range(2):
                        scp = psp.tile(
                            [128, 2, tch], F32, tag="sc", name=f"qk{tch_i}_{pi}"
                        )
                        for ft in range(HPC):
                            fsl = slice(ft * 128, (ft + 1) * 128)
                            for di in range(dt_):
                                nc.tensor.matmul(
                                    scp[:, ft, :],
                                    w_sbs[pi][:, di, fsl],
                                    x_sb[:, di, :],
                                    start=(di == 0),
                                    stop=(di == dt_ - 1),
                                )
                        for ft in range(HPC):
                            evac_qk(pi, ft, scp[:, ft, :], bi, off)
                    for ft in range(HPC):
                        fsl = slice(ft * 128, (ft + 1) * 128)
                        vps = psp.tile(
                            [128, tch], F32, tag="pv", name=f"v{tch_i}_{ft}"
                        )
                        for di in range(dt_):
                            nc.tensor.matmul(
                                vps,
                                w_sbs[2][:, di, fsl],
                                x_sb[:, di, :],
                                start=(di == 0),
                                stop=(di == dt_ - 1),
                            )
                        evac_v(ft, vps, bi, tch_i)

                    # prefetch batch-0 q/k/v into the pre-opened s2in pool
                    # (disjoint SBUF: no WAR wait on stage-1 buffers at the
                    # stage boundary)
                    if tch_i == cpb - 1:
                        for h in range(HPC):
                            pq = s2in.tile([128, t], BF16, tag="q", name=f"pq{h}")
                            nc.sync.dma_start(out=pq, in_=q_scr[h][0][:, :])
                            pk = s2in.tile([128, t], BF16, tag="k", name=f"pk{h}")
                            nc.sync.dma_start(out=pk, in_=k_scr[h][0][:, :])
                            pv_ = s2in.tile(
                                [128, kt, 128], BF16, tag="v", name=f"pv{h}"
                            )
                            nc.sync.dma_start(
                                out=pv_,
                                in_=v_scr[h][0][:, :, :].rearrange(
                                    "tt p dh -> p tt dh"
                                ),
                            )
                            prefetched[h] = (pq, pk, pv_)

            tpool.release()
            xpool.release()
            wpool.release()

            # ======== stage 2+3: attention + interleaved out-projection ====
            with (
                tc.tile_pool(name="s2", bufs=2) as s2pool,
                tc.tile_pool(name="s2e", bufs=5) as epool,
                tc.tile_pool(name="s3w", bufs=1) as wopool,
                tc.tile_pool(name="s3o", bufs=6) as s3pool,
            ):
                wo_sb = wopool.tile([128, HPC, d_model], BF16, tag="wo")
                nc.scalar.dma_start(
                    out=wo_sb,
                    in_=woT[:, :].rearrange("(ft p) d -> p ft d", p=128),
                )

                # pending out-projection groups: (attn_tile, bi, c4, do)
                s3_pending = []
                s3_count = [0]
                drain_mode = [False]

                def emit_s3_group():
                    if not s3_pending:
                        return
                    attn_src, bi_src, c4, do = s3_pending.pop(0)
                    off = c4 * tch
                    gsl = slice(bi_src * t + off, bi_src * t + off + tch)
                    ps = psp.tile([128, tch], F32, tag="s3", name="s3ps")
                    for ft in range(HPC):
                        nc.tensor.matmul(
                            ps,
                            wo_sb[:, ft, do * 128 : (do + 1) * 128],
                            attn_src[:, ft, off : off + tch],
                            start=(ft == 0),
                            stop=(ft == HPC - 1),
                        )
                    osb = s3pool.tile([128, tch], BF16, tag="o")
                    if s3_count[0] % 4 == 3:
                        nc.scalar.copy(osb, ps)
                    else:
                        nc.vector.tensor_copy(osb, ps)
                    if drain_mode[0]:
                        ring = nc.sync if s3_count[0] % 2 == 0 else nc.gpsimd
                    else:
                        ring = nc.gpsimd
                    s3_count[0] += 1
                    ring.dma_start(
                        out=outP[do * 128 : (do + 1) * 128, gsl], in_=osb
                    )

                for bi in range(b):
                    attn_n = s2pool.tile([128, HPC, t], BF16, tag="an")
                    for h in range(HPC):
                        if bi == 0:
                            q_sb, k_sb, v_sb = prefetched[h]
                        else:
                            q_sb = s2in.tile([128, t], BF16, tag="q")
                            nc.sync.dma_start(out=q_sb, in_=q_scr[h][bi][:, :])
                            k_sb = s2in.tile([128, t], BF16, tag="k")
                            nc.sync.dma_start(out=k_sb, in_=k_scr[h][bi][:, :])
                            v_sb = s2in.tile([128, kt, 128], BF16, tag="v")
                            nc.sync.dma_start(
                                out=v_sb,
                                in_=v_scr[h][bi][:, :, :].rearrange(
                                    "tt p dh -> p tt dh"
                                ),
                            )
                        pending_mul = []
                        for qc in range(nqc):
                            qsl = slice(qc * qch, (qc + 1) * qch)
                            e_pairs = [None] * npr

                            def emit_pair(p):
                                sps = psp.tile(
                                    [128, 2, qch], F32, tag="sc", name="sps"
                                )
                                for j in range(2):
                                    kti = 2 * p + j
                                    nc.tensor.matmul(
                                        sps[:, j, :],
                                        k_sb[:, kti * 128 : (kti + 1) * 128],
                                        q_sb[:, qsl],
                                        start=True,
                                        stop=True,
                                    )
                                e_p = epool.tile(
                                    [128, 2, qch], BF16, tag="E", name="e_p"
                                )
                                # one EXP over both k-tiles (1024 cols):
                                # amortizes the 352-cycle ACT overhead
                                nc.scalar.activation(e_p, sps, EXP, scale=SCALE)
                                e_pairs[p] = e_p

                            emit_pair(0)
                            emit_pair(1)
                            while pending_mul:
                                m_pv, m_rcp, m_sl = pending_mul.pop(0)
                                nc.vector.tensor_mul(
                                    attn_n[:, h, m_sl], m_pv, m_rcp
                                )
                            pv = psp.tile([128, qch], F32, tag="pv", name="pv")
                            acc = s2pool.tile(
                                [128, qch], BF16, tag="acc", name="acc"
                            )
                            for p in range(npr):
                                for j in range(2):
                                    nc.tensor.matmul(
                                        pv,
                                        v_sb[:, 2 * p + j, :],
                                        e_pairs[p][:, j, :],
                                        start=(p == 0 and j == 0),
                                        stop=(p == npr - 1 and j == 1),
                                    )
                                # denominator: bf16 pair-collapse (gpsimd)
                                # + running bf16 acc (vector)
                                if p == 0:
                                    nc.vector.tensor_add(
                                        acc,
                                        e_pairs[0][:, 0, :],
                                        e_pairs[0][:, 1, :],
                                    )
                                else:
                                    sp = s2pool.tile(
                                        [128, qch], BF16, tag="sp", name="sp"
                                    )
                                    nc.vector.tensor_add(
                                        sp,
                                        e_pairs[p][:, 0, :],
                                        e_pairs[p][:, 1, :],
                                    )
                                    nc.vector.tensor_add(acc, acc, sp)
                                if p + 2 < npr:
                                    emit_pair(p + 2)
                                emit_s3_group()
                            # partition-broadcast the denominator with one
                            # ones-matmul, then one Newton step for 1/den
                            dnb = psp.tile([1, qch], F32, tag="s3", name="dnb")
                            nc.tensor.matmul(
                                dnb, ones_sb[:, 0:1], acc, start=True, stop=True
                            )
                            # exact 1/den: copy the [1,qch] row out, bounce
                            # it through DRAM to a [128,4] layout, take the
                            # reciprocal on full lanes, bounce back as a
                            # 0-stride broadcast read. ~5us of DMA latency,
                            # fully hidden: attn_n[qc] is first consumed by
                            # the out-projection a chunk later.
                            drow = s2pool.tile([1, qch], F32, tag="drow", name="drow")
                            nc.vector.tensor_copy(drow, dnb)
                            nc.gpsimd.dma_start(
                                out=dnrow_d[h % 2, qc : qc + 1, :], in_=drow
                            )
                            rsm = s2pool.tile(
                                [128, qch // 128], F32, tag="rsm", name="rsm"
                            )
                            nc.gpsimd.dma_start(
                                out=rsm,
                                in_=dnrow_d[h % 2, qc, :].rearrange(
                                    "(p i) -> p i", p=128
                                ),
                            )
                            nc.vector.reciprocal(rsm, rsm)
                            nc.gpsimd.dma_start(
                                out=rcp_d[h % 2, qc, :].rearrange(
                                    "(p i) -> p i", p=128
                                ),
                                in_=rsm,
                            )
                            rcp = s2pool.tile(
                                [128, qch], F32, tag="rcp", name="rcp"
                            )
                            rsrc = rcp_d[h % 2, qc : qc + 1, :]
                            bcast = bass.AP(
                                tensor=rsrc.tensor,
                                offset=rsrc.offset,
                                ap=[[0, 128]] + [list(p) for p in rsrc.ap[1:]],
                            )
                            nc.gpsimd.dma_start(out=rcp, in_=bcast)
                            if qc < nqc - 1:
                                pending_mul.append((pv, rcp, qsl))
                            else:
                                nc.vector.tensor_mul(
                                    attn_n[:, h, qsl], pv, rcp
                                )
                            # out-projection chunk qc-1 becomes eligible one
                            # chunk after head 1 normalizes it, so the
                            # reciprocal's DMA-bounce latency stays hidden
                            if h == 1 and qc >= 1:
                                for do in range(dt_):
                                    s3_pending.append((attn_n, bi, qc - 1, do))
                    if h == 1:
                        for cq in (nqc - 1,):
                            for do in range(dt_):
                                s3_pending.append((attn_n, bi, cq, do))
                # drain the final batch's remaining out-projection groups
                drain_mode[0] = True
                while s3_pending:
                    emit_s3_group()
            s2in.release()

    nc.finalize()
    return nc


_module_cache = {}


def _get_module(b, t, d_model, n_cores):
    key = (b, t, d_model, n_cores)
    if key not in _module_cache:
        _module_cache[key] = build_module(b, t, d_model, n_cores)
    return _module_cache[key]


def _host_tables(t):
    half = HEAD_DIM // 2
    theta = 1.0 / (
        np.float32(ROPE_BASE)
        ** (np.arange(half, dtype=np.float32) / np.float32(half))
    )
    freqs = np.arange(t, dtype=np.float32)[:, None] * theta[None, :]
    emb = np.concatenate([freqs, freqs], axis=-1)  # (t, 128)
    cosT = np.ascontiguousarray(np.cos(emb).T.astype(np.float32))
    sinT = np.sin(emb).T.astype(np.float32)
    rsinT = sinT.copy()
    rsinT[:half] = -sinT[:half]
    rsinT = np.ascontiguousarray(rsinT)
    return cosT, rsinT


def _run(x, Wq, Wk, Wv, Wo, trace=False):
    import ml_dtypes

    bf16 = ml_dtypes.bfloat16
    b_, t_, d_ = x.shape
    n_cores = (d_ // HEAD_DIM) // HPC
    nc = _get_module(b_, t_, d_, n_cores)

    xT = np.ascontiguousarray(x.reshape(b_ * t_, d_).T).astype(bf16)
    cosT, rsinT = _host_tables(t_)
    ident = np.eye(128, dtype=np.float32).astype(bf16)
    onesq = np.ones((128, 128), dtype=np.float32).astype(bf16)

    in_maps = []
    for c in range(n_cores):
        fs = slice(c * F_LOC, (c + 1) * F_LOC)
        in_maps.append(
            {
                "xT": xT,
                "wqT": np.ascontiguousarray(Wq[fs, :].T).astype(bf16),
                "wkT": np.ascontiguousarray(Wk[fs, :].T).astype(bf16),
                "wvT": np.ascontiguousarray(Wv[fs, :].T).astype(bf16),
                "woT": np.ascontiguousarray(Wo[:, fs].T).astype(bf16),
                "cosT": cosT,
                "rsinT": rsinT,
                "ident": ident,
                "onesq": onesq,
            }
        )
    res = run_bass_kernel_spmd(
        nc, in_maps, core_ids=list(range(n_cores)), trace=trace
    )
    acc = res.results[0]["outP"].astype(np.float32)
    for c in range(1, n_cores):
        acc += res.results[c]["outP"].astype(np.float32)
    out = np.ascontiguousarray(acc.T).reshape(b_, t_, d_)
    return out, res


def kernel(x, Wq, Wk, Wv, Wo):
    x = np.asarray(x, dtype=np.float32)
    Wq = np.asarray(Wq, dtype=np.float32)
    Wk = np.asarray(Wk, dtype=np.float32)
    Wv = np.asarray(Wv, dtype=np.float32)
    Wo = np.asarray(Wo, dtype=np.float32)
    out, _ = _run(x, Wq, Wk, Wv, Wo, trace=False)
    return out


if __name__ == "__main__":
    build_module()
    print("module built ok")


# revision 23
# speedup vs baseline: 1.1027x; 1.0257x over previous
"""Trainium2 Bass kernel: 16-head RoPE attention block (B=4, T=2048, D=2048).

Sharding: tensor-parallel over heads. Each of the 8 cores owns 2 heads
(a 256-wide slice of the q/k/v projection output features). Per core:

  stage 1: q/k/v projections in feature-major layout, all-bf16 matmuls
           (x and W arrive bf16: halves DMA + SBUF traffic at the same
           1-col/cycle PE rate), RoPE on the vector engine (bf16 out),
           v transposed to token-major via the PE; results staged in
           DRAM per (head, batch). Chunk 0 is quarter-reordered (q/k
           groups track the x quarters as they land; v groups follow)
           with the loads spread over the three DMA rings.
  stage 2: per (batch, head): scores computed TRANSPOSED (S^T[k,q] =
           kTile^T @ qT, bf16) in PAIRS of k-tiles sharing one two-bank
           PSUM tile so each scalar-engine EXP covers 1024 columns (the
           352-cycle ACT overhead amortizes and the scalar engine stays
           below the PE; only the exp table set is ever loaded). The
           softmax denominator accumulates as bf16 pair-sums + a running
           bf16 acc on the vector engine, is collapsed to a [1,qch] row
           by a ones-column matmul, bounced through DRAM into a [128,4]
           layout for a full-lane DVE reciprocal, and broadcast back
           with a 0-stride DMA read. The ~10us of bounce latency is
           hidden: out-projection work referencing a chunk is deferred
           by one query chunk.
  stage 3: out-projection (bf16 x bf16) matmuls INTERLEAVED into the
           attention loop one query-chunk later, filling the PE's
           exp-wait gaps; evacuated to bf16 (3:1 vector/scalar split)
           and DMAed out on the sync ring.

Host sums the 8 bf16 partial outputs (the "all-reduce") in f32 and
un-transposes. PSUM budget: score pairs 2x2 banks + pv 2 + s3/dnrow 2
= 8 banks. Stage-2 input q/k/v tiles live in a right-side SBUF pool
that outlives stage 1, so batch 0 prefetches with no WAR stall at the
stage boundary. The attn_n normalize-multiply is itself deferred into
the next chunk's pair loop so the in-order DVE queue never blocks on
the bounce DMA. Measured: 872.9us, rel err 5.9e-3 (vs 1067.1us
baseline).
"""

import math

import numpy as np

import concourse.bacc as bacc
import concourse.bass as bass
import concourse.mybir as mybir
import concourse.tile as tile
from concourse.bass_utils import run_bass_kernel_spmd

F32 = mybir.dt.float32
F32R = mybir.dt.float32r
BF16 = mybir.dt.bfloat16
EXP = mybir.ActivationFunctionType.Exp
LN = mybir.ActivationFunctionType.Ln

# Problem shape (hardcoded; the harness calls kernel() with exactly these).
B = 4
T = 2048
D_MODEL = 2048
HEAD_DIM = 128
N_CORES = 8
ROPE_BASE = 10000.0

HPC = 2                      # heads per core
F_LOC = HPC * HEAD_DIM       # 256 local projection features per core
BT = B * T
TCH = 512                    # token chunk width (stages 1/3)
QCH = 512                    # query chunk width (stage 2)
SCALE = 1.0 / math.sqrt(HEAD_DIM)


def build_module(b=B, t=T, d_model=D_MODEL, n_cores=N_CORES):
    """Build the per-core Bass module. All cores run the same program on
    different data (pure SPMD, no collectives)."""
    bt = b * t
    dt_ = d_model // 128
    kt = t // 128
    npr = kt // 2            # k-tile pairs per query chunk
    tch = min(TCH, bt)
    qch = min(QCH, t)
    ntch = bt // tch
    nqc = t // qch
    cpb = t // tch           # stage-1/3 token chunks per batch
    qd = dt_ // 4            # d-tile quarter for the startup loads

    nc = bacc.Bacc(None, target_bir_lowering=False)

    xT = nc.dram_tensor("xT", [d_model, bt], BF16, kind="ExternalInput")
    wqT = nc.dram_tensor("wqT", [d_model, F_LOC], BF16, kind="ExternalInput")
    wkT = nc.dram_tensor("wkT", [d_model, F_LOC], BF16, kind="ExternalInput")
    wvT = nc.dram_tensor("wvT", [d_model, F_LOC], BF16, kind="ExternalInput")
    woT = nc.dram_tensor("woT", [F_LOC, d_model], BF16, kind="ExternalInput")
    cosT = nc.dram_tensor("cosT", [HEAD_DIM, t], F32, kind="ExternalInput")
    rsinT = nc.dram_tensor("rsinT", [HEAD_DIM, t], F32, kind="ExternalInput")
    ident = nc.dram_tensor("ident", [128, 128], BF16, kind="ExternalInput")
    onesq = nc.dram_tensor("onesq", [128, 128], BF16, kind="ExternalInput")
    outP = nc.dram_tensor("outP", [d_model, bt], BF16, kind="ExternalOutput")

    with tile.TileContext(nc) as tc:
        with (
            tc.tile_pool(name="const", bufs=1) as constp,
            tc.tile_pool(name="dram", bufs=1, space="DRAM") as dram,
            tc.tile_pool(name="psp", bufs=2, space="PSUM") as psp,
        ):
            # constants: tiles here, DMAs emitted inside stage 1 so the
            # per-ring issue order puts the matmul-gating loads first
            cos_sb = constp.tile([128, t], F32)
            rsin_sb = constp.tile([128, t], F32)
            id_sb = constp.tile([128, 128], BF16)
            ones_sb = constp.tile([128, 128], BF16)

            # DRAM scratch, per (head, batch): cross-stage deps stay
            # batch-granular so the stages pipeline
            q_scr = [
                [dram.tile([128, t], BF16, name=f"qs{h}_{bi}", tag=f"qs{h}_{bi}") for bi in range(b)]
                for h in range(HPC)
            ]
            k_scr = [
                [dram.tile([128, t], BF16, name=f"ks{h}_{bi}", tag=f"ks{h}_{bi}") for bi in range(b)]
                for h in range(HPC)
            ]
            v_scr = [
                [dram.tile([kt, 128, 128], BF16, name=f"vs{h}_{bi}", tag=f"vs{h}_{bi}") for bi in range(b)]
                for h in range(HPC)
            ]

            prefetched = {}
            # per-qc scratch rows for the DMA-reshaped reciprocal
            dnrow_d = dram.tile([2, t // 128 // 4, 512], F32, name="dnrow_d")
            rcp_d = dram.tile([2, t // 128 // 4, 512], F32, name="rcp_d")

            # ================= stage 1: projections + rope + v^T =========
            # manual pool lifetimes: the stage-1 pools take the low SBUF
            # addresses; s2in is allocated above them and outlives stage 1
            # (the batch-0 q/k/v prefetch needs no WAR wait on stage-1
            # buffers at the stage boundary)
            wpool = tc.alloc_tile_pool(name="s1w", bufs=1)
            xpool = tc.alloc_tile_pool(name="s1x", bufs=2)
            tpool = tc.alloc_tile_pool(name="s1t", bufs=4)
            s2in = tc.alloc_tile_pool(name="s2in", bufs=3, side="right")
            if True:
                w_sbs = []
                wsrcs = []
                for wten, wname in ((wqT, "wq"), (wkT, "wk"), (wvT, "wv")):
                    wsb = wpool.tile([128, dt_, F_LOC], BF16, tag=wname)
                    w_sbs.append(wsb)
                    wsrcs.append(
                        wten[:, :].rearrange("(dt p) f -> p dt f", p=128)
                    )

                # ---- startup loads over the three DMA rings ----
                x0 = xpool.tile([128, dt_, tch], BF16, tag="x")
                x0src = xT[:, 0:tch].rearrange("(dt p) tt -> p dt tt", p=128)
                # sync ring: the four x chunk-0 quarters (chunks 2+ follow)
                for qtr in range(4):
                    dsl = slice(qtr * qd, (qtr + 1) * qd)
                    nc.sync.dma_start(out=x0[:, dsl, :], in_=x0src[:, dsl, :])
                # scalar ring: wq quarters (x chunk 1 + wo follow)
                for qtr in range(4):
                    dsl = slice(qtr * qd, (qtr + 1) * qd)
                    nc.scalar.dma_start(
                        out=w_sbs[0][:, dsl, :], in_=wsrcs[0][:, dsl, :]
                    )
                # gpsimd ring: wk halves, wv halves, then constants
                for wi in (1, 2):
                    for hf in range(2):
                        dsl = slice(hf * 2 * qd, (hf + 1) * 2 * qd)
                        nc.gpsimd.dma_start(
                            out=w_sbs[wi][:, dsl, :], in_=wsrcs[wi][:, dsl, :]
                        )
                nc.gpsimd.dma_start(out=cos_sb, in_=cosT[:, :])
                nc.gpsimd.dma_start(out=rsin_sb, in_=rsinT[:, :])
                nc.gpsimd.dma_start(out=id_sb, in_=ident[:, :])
                nc.gpsimd.dma_start(out=ones_sb, in_=onesq[:, :])

                def evac_qk(pi, ft, ps, bi, off):
                    """RoPE + bf16 store for one q/k psum group."""
                    lsl = slice(off, off + tch)
                    ro = tpool.tile([128, tch], F32, tag="ro")
                    nc.vector.tensor_mul(ro, ps, cos_sb[:, lsl])
                    rt = tpool.tile([128, tch], F32, tag="rt")
                    nc.vector.tensor_mul(rt[0:64], ps[64:128], rsin_sb[0:64, lsl])
                    nc.vector.tensor_mul(rt[64:128], ps[0:64], rsin_sb[64:128, lsl])
                    rs = tpool.tile([128, tch], BF16, tag="rs")
                    nc.vector.tensor_add(rs, ro, rt)
                    scr = q_scr if pi == 0 else k_scr
                    nc.gpsimd.dma_start(out=scr[ft][bi][:, lsl], in_=rs)

                def evac_v(ft, ps, bi, tch_i):
                    """bf16 copy + PE transpose + store for one v group."""
                    vsb = tpool.tile([128, tch], BF16, tag="vs")
                    nc.scalar.copy(vsb, ps)
                    for j in range(tch // 128):
                        pst = psp.tile([128, 128], BF16, tag="s3", name="pst")
                        nc.tensor.transpose(
                            pst, vsb[:, j * 128 : (j + 1) * 128], id_sb
                        )
                        vt = tpool.tile([128, 128], BF16, tag="vt")
                        nc.vector.tensor_copy(vt, pst)
                        nc.gpsimd.dma_start(
                            out=v_scr[ft][bi][
                                (tch_i % cpb) * (tch // 128) + j, :, :
                            ],
                            in_=vt,
                        )

                # ---- chunk 0: q/k groups follow the x quarters; v after ----
                qk_pairs = []
                for pi in range(2):
                    scp = psp.tile([128, 2, tch], F32, tag="sc", name=f"c0qk{pi}")
                    qk_pairs.append(scp)
                v_ps = []
                for ft in range(HPC):
                    vps = psp.tile([128, tch], F32, tag="pv", name=f"c0v{ft}")
                    v_ps.append(vps)
                for dq in range(4):
                    for pi in range(2):
                        for ft in range(HPC):
                            fsl = slice(ft * 128, (ft + 1) * 128)
                            for di in range(dq * qd, (dq + 1) * qd):
                                nc.tensor.matmul(
                                    qk_pairs[pi][:, ft, :],
                                    w_sbs[pi][:, di, fsl],
                                    x0[:, di, :],
                                    start=(di == 0),
                                    stop=(di == dt_ - 1),
                                )
                for ft in range(HPC):
                    fsl = slice(ft * 128, (ft + 1) * 128)
                    for di in range(dt_):
                        nc.tensor.matmul(
                            v_ps[ft],
                            w_sbs[2][:, di, fsl],
                            x0[:, di, :],
                            start=(di == 0),
                            stop=(di == dt_ - 1),
                        )
                for pi in range(2):
                    for ft in range(HPC):
                        evac_qk(pi, ft, qk_pairs[pi][:, ft, :], 0, 0)
                for ft in range(HPC):
                    evac_v(ft, v_ps[ft], 0, 0)

                # ---- chunks 1..ntch-1 ----
                for tch_i in range(1, ntch):
                    bi = tch_i // cpb
                    off = (tch_i % cpb) * tch
                    tsl = slice(tch_i * tch, (tch_i + 1) * tch)
                    x_sb = xpool.tile([128, dt_, tch], BF16, tag="x")
                    xsrc = xT[:, tsl].rearrange("(dt p) tt -> p dt tt", p=128)
                    # chunk 1 rides the scalar ring (free after wq) so it
                    # lands before chunk 0's compute finishes
                    ring = nc.scalar if tch_i == 1 else nc.sync
                    ring.dma_start(out=x_sb, in_=xsrc)
                    for pi in range(2):
                        scp = psp.tile(
                            [128, 2, tch], F32, tag="sc", name=f"qk{tch_i}_{pi}"
                        )
                        for ft in range(HPC):
                            fsl = slice(ft * 128, (ft + 1) * 128)
                            for di in range(dt_):
                                nc.tensor.matmul(
                                    scp[:, ft, :],
                                    w_sbs[pi][:, di, fsl],
                                    x_sb[:, di, :],
                                    start=(di == 0),
                                    stop=(di == dt_ - 1),
                                )
                        for ft in range(HPC):
                            evac_qk(pi, ft, scp[:, ft, :], bi, off)
                    for ft in range(HPC):
                        fsl = slice(ft * 128, (ft + 1) * 128)
                        vps = psp.tile(
                            [128, tch], F32, tag="pv", name=f"v{tch_i}_{ft}"
                        )
                        for di in range(dt_):
                            nc.tensor.matmul(
                                vps,
                                w_sbs[2][:, di, fsl],
                                x_sb[:, di, :],
                                start=(di == 0),
                                stop=(di == dt_ - 1),
                            )
                        evac_v(ft, vps, bi, tch_i)

                    # prefetch batch-0 q/k/v into the pre-opened s2in pool
                    # (disjoint SBUF: no WAR wait on stage-1 buffers at the
                    # stage boundary)
                    if tch_i == cpb - 1:
                        for h in range(HPC):
                            pq = s2in.tile([128, t], BF16, tag="q", name=f"pq{h}")
                            nc.sync.dma_start(out=pq, in_=q_scr[h][0][:, :])
                            pk = s2in.tile([128, t], BF16, tag="k", name=f"pk{h}")
                            nc.sync.dma_start(out=pk, in_=k_scr[h][0][:, :])
                            pv_ = s2in.tile(
                                [128, kt, 128], BF16, tag="v", name=f"pv{h}"
                            )
                            nc.sync.dma_start(
                                out=pv_,
                                in_=v_scr[h][0][:, :, :].rearrange(
                                    "tt p dh -> p tt dh"
                                ),
                            )
                            prefetched[h] = (pq, pk, pv_)

            tpool.release()
            xpool.release()
            wpool.release()

            # ======== stage 2+3: attention + interleaved out-projection ====
            with (
                tc.tile_pool(name="s2", bufs=2) as s2pool,
                tc.tile_pool(name="s2e", bufs=5) as epool,
                tc.tile_pool(name="s3w", bufs=1) as wopool,
                tc.tile_pool(name="s3o", bufs=6) as s3pool,
            ):
                wo_sb = wopool.tile([128, HPC, d_model], BF16, tag="wo")
                nc.scalar.dma_start(
                    out=wo_sb,
                    in_=woT[:, :].rearrange("(ft p) d -> p ft d", p=128),
                )

                # pending out-projection groups: (attn_tile, bi, c4, do)
                s3_pending = []
                s3_count = [0]

                def emit_s3_group():
                    if not s3_pending:
                        return
                    attn_src, bi_src, c4, do = s3_pending.pop(0)
                    off = c4 * tch
                    gsl = slice(bi_src * t + off, bi_src * t + off + tch)
                    ps = psp.tile([128, tch], F32, tag="s3", name="s3ps")
                    for ft in range(HPC):
                        nc.tensor.matmul(
                            ps,
                            wo_sb[:, ft, do * 128 : (do + 1) * 128],
                            attn_src[:, ft, off : off + tch],
                            start=(ft == 0),
                            stop=(ft == HPC - 1),
                        )
                    osb = s3pool.tile([128, tch], BF16, tag="o")
                    if s3_count[0] % 4 == 3:
                        nc.scalar.copy(osb, ps)
                    else:
                        nc.vector.tensor_copy(osb, ps)
                    ring = nc.sync if s3_count[0] % 2 == 0 else nc.gpsimd
                    s3_count[0] += 1
                    ring.dma_start(
                        out=outP[do * 128 : (do + 1) * 128, gsl], in_=osb
                    )

                def load_pair(bi_, h_):
                    lq = s2in.tile([128, t], BF16, tag="q", name=f"lq{bi_}{h_}")
                    nc.sync.dma_start(out=lq, in_=q_scr[h_][bi_][:, :])
                    lk = s2in.tile([128, t], BF16, tag="k", name=f"lk{bi_}{h_}")
                    nc.sync.dma_start(out=lk, in_=k_scr[h_][bi_][:, :])
                    lv = s2in.tile(
                        [128, kt, 128], BF16, tag="v", name=f"lv{bi_}{h_}"
                    )
                    nc.sync.dma_start(
                        out=lv,
                        in_=v_scr[h_][bi_][:, :, :].rearrange(
                            "tt p dh -> p tt dh"
                        ),
                    )
                    return (lq, lk, lv)

                loaded = {(0, h): prefetched[h] for h in range(HPC)}
                for bi in range(b):
                    attn_n = s2pool.tile([128, HPC, t], BF16, tag="an")
                    for h in range(HPC):
                        # issue the NEXT pair's loads a full head (~37us)
                        # early so the strided v read never lands on a
                        # head boundary
                        nxt = (bi, h + 1) if h + 1 < HPC else (bi + 1, 0)
                        if nxt[0] < b and nxt not in loaded:
                            loaded[nxt] = load_pair(*nxt)
                        q_sb, k_sb, v_sb = loaded.pop((bi, h))
                        pending_mul = []
                        for qc in range(nqc):
                            qsl = slice(qc * qch, (qc + 1) * qch)
                            e_pairs = [None] * npr

                            def emit_pair(p):
                                sps = psp.tile(
                                    [128, 2, qch], F32, tag="sc", name="sps"
                                )
                                for j in range(2):
                                    kti = 2 * p + j
                                    nc.tensor.matmul(
                                        sps[:, j, :],
                                        k_sb[:, kti * 128 : (kti + 1) * 128],
                                        q_sb[:, qsl],
                                        start=True,
                                        stop=True,
                                    )
                                e_p = epool.tile(
                                    [128, 2, qch], BF16, tag="E", name="e_p"
                                )
                                # one EXP over both k-tiles (1024 cols):
                                # amortizes the 352-cycle ACT overhead
                                nc.scalar.activation(e_p, sps, EXP, scale=SCALE)
                                e_pairs[p] = e_p

                            emit_pair(0)
                            emit_pair(1)
                            while pending_mul:
                                m_pv, m_rcp, m_sl = pending_mul.pop(0)
                                nc.vector.tensor_mul(
                                    attn_n[:, h, m_sl], m_pv, m_rcp
                                )
                            pv = psp.tile([128, qch], F32, tag="pv", name="pv")
                            acc = s2pool.tile(
                                [128, qch], BF16, tag="acc", name="acc"
                            )
                            for p in range(npr):
                                for j in range(2):
                                    nc.tensor.matmul(
                                        pv,
                                        v_sb[:, 2 * p + j, :],
                                        e_pairs[p][:, j, :],
                                        start=(p == 0 and j == 0),
                                        stop=(p == npr - 1 and j == 1),
                                    )
                                # denominator: bf16 pair-collapse (gpsimd)
                                # + running bf16 acc (vector)
                                if p == 0:
                                    nc.vector.tensor_add(
                                        acc,
                                        e_pairs[0][:, 0, :],
                                        e_pairs[0][:, 1, :],
                                    )
                                else:
                                    sp = s2pool.tile(
                                        [128, qch], BF16, tag="sp", name="sp"
                                    )
                                    nc.vector.tensor_add(
                                        sp,
                                        e_pairs[p][:, 0, :],
                                        e_pairs[p][:, 1, :],
                                    )
                                    nc.vector.tensor_add(acc, acc, sp)
                                if p + 2 < npr:
                                    emit_pair(p + 2)
                                emit_s3_group()
                            # partition-broadcast the denominator with one
                            # ones-matmul, then one Newton step for 1/den
                            dnb = psp.tile([1, qch], F32, tag="s3", name="dnb")
                            nc.tensor.matmul(
                                dnb, ones_sb[:, 0:1], acc, start=True, stop=True
                            )
                            # exact 1/den: copy the [1,qch] row out, bounce
                            # it through DRAM to a [128,4] layout, take the
                            # reciprocal on full lanes, bounce back as a
                            # 0-stride broadcast read. ~5us of DMA latency,
                            # fully hidden: attn_n[qc] is first consumed by
                            # the out-projection a chunk later.
                            drow = s2pool.tile([1, qch], F32, tag="drow", name="drow")
                            nc.vector.tensor_copy(drow, dnb)
                            nc.gpsimd.dma_start(
                                out=dnrow_d[h % 2, qc : qc + 1, :], in_=drow
                            )
                            rsm = s2pool.tile(
                                [128, qch // 128], F32, tag="rsm", name="rsm"
                            )
                            nc.gpsimd.dma_start(
                                out=rsm,
                                in_=dnrow_d[h % 2, qc, :].rearrange(
                                    "(p i) -> p i", p=128
                                ),
                            )
                            nc.vector.reciprocal(rsm, rsm)
                            nc.gpsimd.dma_start(
                                out=rcp_d[h % 2, qc, :].rearrange(
                                    "(p i) -> p i", p=128
                                ),
                                in_=rsm,
                            )
                            rcp = s2pool.tile(
                                [128, qch], F32, tag="rcp", name="rcp"
                            )
                            rsrc = rcp_d[h % 2, qc : qc + 1, :]
                            bcast = bass.AP(
                                tensor=rsrc.tensor,
                                offset=rsrc.offset,
                                ap=[[0, 128]] + [list(p) for p in rsrc.ap[1:]],
                            )
                            nc.gpsimd.dma_start(out=rcp, in_=bcast)
                            if qc < nqc - 1:
                                pending_mul.append((pv, rcp, qsl))
                            else:
                                nc.vector.tensor_mul(
                                    attn_n[:, h, qsl], pv, rcp
                                )
                            # out-projection chunk qc-1 becomes eligible one
                            # chunk after head 1 normalizes it, so the
                            # reciprocal's DMA-bounce latency stays hidden
                            if h == 1 and qc >= 1:
                                for do in range(dt_):
                                    s3_pending.append((attn_n, bi, qc - 1, do))
                    if h == 1:
                        for cq in (nqc - 1,):
                            for do in range(dt_):
                                s3_pending.append((attn_n, bi, cq, do))
                # drain the final batch's remaining out-projection groups
                while s3_pending:
                    emit_s3_group()
            s2in.release()

    nc.finalize()
    return nc


_module_cache = {}


def _get_module(b, t, d_model, n_cores):
    key = (b, t, d_model, n_cores)
    if key not in _module_cache:
        _module_cache[key] = build_module(b, t, d_model, n_cores)
    return _module_cache[key]


def _host_tables(t):
    half = HEAD_DIM // 2
    theta = 1.0 / (
        np.float32(ROPE_BASE)
        ** (np.arange(half, dtype=np.float32) / np.float32(half))
    )
    freqs = np.arange(t, dtype=np.float32)[:, None] * theta[None, :]
    emb = np.concatenate([freqs, freqs], axis=-1)  # (t, 128)
    cosT = np.ascontiguousarray(np.cos(emb).T.astype(np.float32))
    sinT = np.sin(emb).T.astype(np.float32)
    rsinT = sinT.copy()
    rsinT[:half] = -sinT[:half]
    rsinT = np.ascontiguousarray(rsinT)
    return cosT, rsinT


def _run(x, Wq, Wk, Wv, Wo, trace=False):
    import ml_dtypes

    bf16 = ml_dtypes.bfloat16
    b_, t_, d_ = x.shape
    n_cores = (d_ // HEAD_DIM) // HPC
    nc = _get_module(b_, t_, d_, n_cores)

    xT = np.ascontiguousarray(x.reshape(b_ * t_, d_).T).astype(bf16)
    cosT, rsinT = _host_tables(t_)
    ident = np.eye(128, dtype=np.float32).astype(bf16)
    onesq = np.ones((128, 128), dtype=np.float32).astype(bf16)

    in_maps = []
    for c in range(n_cores):
        fs = slice(c * F_LOC, (c + 1) * F_LOC)
        in_maps.append(
            {
                "xT": xT,
                "wqT": np.ascontiguousarray(Wq[fs, :].T).astype(bf16),
                "wkT": np.ascontiguousarray(Wk[fs, :].T).astype(bf16),
                "wvT": np.ascontiguousarray(Wv[fs, :].T).astype(bf16),
                "woT": np.ascontiguousarray(Wo[:, fs].T).astype(bf16),
                "cosT": cosT,
                "rsinT": rsinT,
                "ident": ident,
                "onesq": onesq,
            }
        )
    res = run_bass_kernel_spmd(
        nc, in_maps, core_ids=list(range(n_cores)), trace=trace
    )
    acc = res.results[0]["outP"].astype(np.float32)
    for c in range(1, n_cores):
        acc += res.results[c]["outP"].astype(np.float32)
    out = np.ascontiguousarray(acc.T).reshape(b_, t_, d_)
    return out, res


def kernel(x, Wq, Wk, Wv, Wo):
    x = np.asarray(x, dtype=np.float32)
    Wq = np.asarray(Wq, dtype=np.float32)
    Wk = np.asarray(Wk, dtype=np.float32)
    Wv = np.asarray(Wv, dtype=np.float32)
    Wo = np.asarray(Wo, dtype=np.float32)
    out, _ = _run(x, Wq, Wk, Wv, Wo, trace=False)
    return out


if __name__ == "__main__":
    build_module()
    print("module built ok")
